# revision 1
# baseline (speedup 1.0000x reference)
"""Bahdanau-style attention kernel for Trainium2 (8 NeuronCores, SPMD).

Math (per batch row b):
    h_proj = hidden @ a_w[:DEC]                       (DEC,)
    e_proj[s, :] = enc[s, :] @ a_w[DEC:]              (S, DEC)
    energy = tanh(e_proj + h_proj + a_b)              (S, DEC)
    scores = energy @ v_w                             (S,)
    scores = where(mask == 0, -1e10, scores)
    attn = softmax(scores)                            (S,)
    out = attn @ enc                                  (ENC,)

Sharding: data-parallel over batch (32 rows -> 4 rows on each of 8 cores);
weights replicated.

Per-core strategy (dense path, SPARSE=False — see note above P_PAD for the
optional on-device mask-compaction path):
  - Encoder outputs are DMA-loaded with an fp32->bf16 cast (SWDGE) in four
    512-token chunks per batch row, kept in natural (s, e) layout for the
    final weighted sum.
  - e_proj is computed transposed (d on partitions, tokens on free dim; the
    (e, tok) operand comes from the DMA xbar transpose) so that
    (h_proj + a_b) is a per-partition scalar -> one ScalarE activation does
    bias + tanh while evacuating PSUM.
  - scores = v . tanh is a K=128 M=1 matmul; the attn row is transposed
    back to columns with K=1 matmuls against a 1x1 ones operand; the
    weighted sum is a K=128(s) M=1 matmul over the natural-layout gathered
    rows (pad rows are zeroed by the compact mask, so they add 0).
All matmuls run in bf16 with fp32 PSUM accumulation (measured end-to-end
scale-relative error ~2e-3 vs the fp32 reference).
"""

import numpy as np
from contextlib import ExitStack

B, S, ENC, DEC = 32, 2048, 1024, 1024
N_CORES = 8
BC = B // N_CORES  # batch rows per core
# padded compact-token count: Binomial(2048, 0.5) is 1024 +- 22.6, so 1152
# is a +5.7 sigma bound on the per-row unmasked count (~1e-8 per row;
# seed-0 data maxes at 1062)
P_PAD = 1152

# The sparse (mask-compaction) path is numerically validated on hardware
# (rel err 2.44e-3, identical to dense) using the HW-correct row-granularity
# scatter (one row index per partition, 16-byte payloads; elementwise and
# multi-index-per-partition scatters scramble on silicon). It cuts TensorE
# work ~36%, but the index build needs ~20 small SWDGE ops per batch row and
# the Q7 descriptor-generation rate (~1-3us per indirect op, serial) makes
# the whole pipeline Pool-bound: cost model 500us vs 352us dense. Dense
# ships; flip SPARSE=True to use the compaction path.
SPARSE = False


def build_bass_kernel(
    bc=BC, s=S, e_dim=ENC, d_dim=DEC, debug=False, sparse=SPARSE, p_pad=None
):
    import concourse.bass as bass
    import concourse.tile as tile
    from concourse import bacc, mybir

    f32 = mybir.dt.float32
    bf16 = mybir.dt.bfloat16
    i32 = mybir.dt.int32
    Tanh = mybir.ActivationFunctionType.Tanh
    Exp = mybir.ActivationFunctionType.Exp
    Alu = mybir.AluOpType

    assert s % 512 == 0 and e_dim % 512 == 0 and d_dim % 128 == 0
    if p_pad is None:
        p_pad = P_PAD if s == 2048 else (s // 2 + 128)
    if not sparse:
        p_pad = s
    assert p_pad % 128 == 0
    n_ct = p_pad // 128            # compact s-tiles per batch row
    # chunk sizes (matmul free dim), each <=512 and a multiple of 128
    chunk_sizes = []
    rem = p_pad
    while rem > 0:
        c = min(512, rem)
        chunk_sizes.append(c)
        rem -= c
    n_chunks = len(chunk_sizes)
    n_et = e_dim // 128            # contraction tiles for e_proj
    n_dt = d_dim // 128            # d (output) tiles for e_proj
    n_ec = e_dim // 512            # 512-wide e chunks for the weighted sum
    # (chunk, within-chunk) of each compact s-tile
    tile_map = []
    for c, csz in enumerate(chunk_sizes):
        for j in range(csz // 128):
            tile_map.append((c, j))

    nc = bacc.Bacc("TRN2", target_bir_lowering=False, debug=debug)

    hs_h = nc.dram_tensor("hidden_states", [bc, d_dim], f32, kind="ExternalInput")
    enc_h = nc.dram_tensor("encoder_outputs", [bc, s, e_dim], f32, kind="ExternalInput")
    msk_h = nc.dram_tensor("encoder_masks", [bc, s], i32, kind="ExternalInput")
    aw_h = nc.dram_tensor("a_w", [e_dim + d_dim, d_dim], f32, kind="ExternalInput")
    ab_h = nc.dram_tensor("a_b", [d_dim], f32, kind="ExternalInput")
    vw_h = nc.dram_tensor("v_w", [d_dim], f32, kind="ExternalInput")
    id_h = nc.dram_tensor("ident", [bc, bc], bf16, kind="ExternalInput")
    if sparse:
        iota_pf_h = nc.dram_tensor("iota_pf", [1, p_pad], f32, kind="ExternalInput")
        tokrep_h = nc.dram_tensor("tokrep", [128, s // 128, 4], i32, kind="ExternalInput")
        iota_ppi_h = nc.dram_tensor("iota_ppi", [128, p_pad // 128], i32, kind="ExternalInput")
    out_h = nc.dram_tensor("out", [bc, e_dim], f32, kind="ExternalOutput")

    enc_flat = enc_h[:, :, :].rearrange("b s e -> (b s) e")

    with tile.TileContext(nc) as tc, ExitStack() as ctx:
        consts = ctx.enter_context(tc.tile_pool(name="consts", bufs=1))
        enc_pool = ctx.enter_context(tc.tile_pool(name="enc", bufs=3 * n_chunks - 1 if sparse else 2 * n_chunks + 2))
        encT_pool = ctx.enter_context(tc.tile_pool(name="encT", bufs=2))
        tanh_pool = ctx.enter_context(tc.tile_pool(name="tanh", bufs=3))
        sm_pool = ctx.enter_context(tc.tile_pool(name="softmax", bufs=2))
        msk_pool = ctx.enter_context(tc.tile_pool(name="mask", bufs=2))
        small_pool = ctx.enter_context(tc.tile_pool(name="small", bufs=4))
        outsb_pool = ctx.enter_context(tc.tile_pool(name="outsb", bufs=1 if sparse else 2))
        pe_psum = ctx.enter_context(tc.tile_pool(name="pe_psum", bufs=2, space="PSUM"))
        sc_psum = ctx.enter_context(tc.tile_pool(name="sc_psum", bufs=2, space="PSUM"))
        at_psum = ctx.enter_context(tc.tile_pool(name="at_psum", bufs=1, space="PSUM"))
        w_psum = ctx.enter_context(tc.tile_pool(name="w_psum", bufs=2, space="PSUM"))
        if sparse:
            dram_pool = ctx.enter_context(
                tc.tile_pool(name="dram", bufs=2, space="DRAM")
            )

        # ---------------- prep: small tensors ----------------
        ident_sb = consts.tile([bc, bc], bf16)
        nc.sync.dma_start(out=ident_sb, in_=id_h[:, :])
        ones_bf = ident_sb[0:1, 0:1]

        hs_bf = consts.tile([bc, d_dim], bf16)
        nc.gpsimd.dma_start(out=hs_bf, in_=hs_h[:, :])  # cast f32->bf16

        v_sb = consts.tile([128, n_dt], bf16)
        nc.gpsimd.dma_start(out=v_sb, in_=vw_h[:].rearrange("(i p) -> p i", p=128))

        ab_sb = consts.tile([128, n_dt], f32)
        nc.sync.dma_start(out=ab_sb, in_=ab_h[:].rearrange("(i p) -> p i", p=128))

        if sparse:
            zeros_f = consts.tile([1, s], f32)
            nc.vector.memset(zeros_f, 0.0)
            iota_cf = consts.tile([1, p_pad], f32)
            nc.sync.dma_start(out=iota_cf, in_=iota_pf_h[:, :])
            tokrep_sb = consts.tile([128, s // 128, 4], i32)
            nc.sync.dma_start(out=tokrep_sb, in_=tokrep_h[:, :, :])
            iota_ppi = consts.tile([128, p_pad // 128], i32)
            nc.sync.dma_start(out=iota_ppi, in_=iota_ppi_h[:, :])
            zeros4 = consts.tile([128, 4], i32)
            nc.vector.memset(zeros4, 0)
            # two alternating DRAM index buffers (4-wide i32 rows; only
            # col 0 is consumed). Zero-init rows 0..p_pad-1 ONCE with the
            # HW-validated scatter shape: one row index per partition,
            # 16-byte row payload. Later batches overwrite the first
            # `count` rows; stale pad rows still hold valid (masked-out)
            # token ids.
            idx_bufs = []
            for nm in ("idxA", "idxB"):
                buf = dram_pool.tile([s, 4], i32, tag=nm)
                for j in range(p_pad // 128):
                    nc.gpsimd.indirect_dma_start(
                        out=buf[:, :],
                        out_offset=bass.IndirectOffsetOnAxis(
                            ap=iota_ppi[:, j : j + 1], axis=0
                        ),
                        in_=zeros4,
                        in_offset=None,
                    )
                idx_bufs.append(buf)

        state = {}

        def emit_loads(b):
            chunks = []
            if sparse:
                # ---- on-device compaction of unmasked token indices ----
                msk_b = msk_pool.tile([1, s], i32, tag="mask")
                nc.sync.dma_start(out=msk_b, in_=msk_h[b : b + 1, :])
                maskf = msk_pool.tile([1, s], f32, tag="maskf")
                nc.vector.tensor_copy(out=maskf, in_=msk_b)
                # inclusive prefix sum of the 0/1 mask
                cums = msk_pool.tile([1, s], f32, tag="cums")
                nc.vector.tensor_tensor_scan(
                    cums, maskf, zeros_f, 0.0, op0=Alu.add, op1=Alu.add
                )
                # compact-lane validity mask (count = last prefix value)
                count_ap = cums[0:1, s - 1 : s]
                maskc = sm_pool.tile([1, p_pad], bf16, tag="maskc")
                nc.vector.tensor_scalar(
                    maskc, iota_cf, count_ap, None, op0=Alu.is_lt
                )
                # compact position for kept tokens, dump row p_pad for
                # masked ones (collisions there are never read):
                # offi = (cums - (1 + p_pad)) * maskf + p_pad
                # (in-place into maskf, then int-cast into cums' bytes --
                # SBUF is tight with two batches of lookahead)
                nc.vector.scalar_tensor_tensor(
                    maskf, cums, -(1.0 + p_pad), maskf, op0=Alu.add, op1=Alu.mult
                )
                offi = cums.bitcast(i32)
                nc.vector.tensor_scalar(
                    offi, maskf, float(p_pad), None, op0=Alu.add
                )
                # round-trip through DRAM to get offsets in (partition, j)
                # layout: the HW scatter wants one row index per partition
                off_d = dram_pool.tile([1, s], i32, tag="offd")
                nc.sync.dma_start(out=off_d, in_=offi)
                offi_pb = msk_pool.tile([128, s // 128], i32, tag="offpb")
                nc.sync.dma_start(
                    out=offi_pb,
                    in_=off_d[0:1, :].rearrange("one (j p) -> p (j one)", p=128),
                )
                # global token ids for this batch row as 16-byte row payloads
                valb = msk_pool.tile([128, s // 128, 4], i32, tag="valb")
                nc.vector.tensor_scalar_add(valb, tokrep_sb, float(b * s))
                idx_d = idx_bufs[b % 2]
                for j in range(s // 128):
                    nc.gpsimd.indirect_dma_start(
                        out=idx_d[:, :],
                        out_offset=bass.IndirectOffsetOnAxis(
                            ap=offi_pb[:, j : j + 1], axis=0
                        ),
                        in_=valb[:, j, :],
                        in_offset=None,
                    )
                idx_sb = msk_pool.tile([128, n_ct, 4], i32, tag="idx_sb")
                nc.sync.dma_start(
                    out=idx_sb,
                    in_=idx_d[0:p_pad, :].rearrange("(j p) r -> p j r", p=128),
                )
                # gather unmasked encoder rows (cast f32->bf16 in the
                # DMA); one (128,1)-index call per compact s-tile — the
                # HW-validated gather shape
                g = 0
                for c, csz in enumerate(chunk_sizes):
                    st_c = csz // 128
                    enc_c = enc_pool.tile([128, 4, e_dim], bf16, tag="enc")
                    for jj in range(st_c):
                        nc.gpsimd.indirect_dma_start(
                            out=enc_c[:, jj, :],
                            out_offset=None,
                            in_=enc_flat,
                            in_offset=bass.IndirectOffsetOnAxis(
                                ap=idx_sb[:, g, 0:1], axis=0
                            ),
                        )
                        g += 1
                    chunks.append(enc_c)
                state[b] = dict(enc=chunks, pmask=maskc)
            else:
                pos = 0
                for t, csz in enumerate(chunk_sizes):
                    if b == 0 and t == 0:
                        chunks.append(enc_b0_c0)
                        pos += csz
                        continue
                    enc_c = enc_pool.tile([128, 4, e_dim], bf16, tag="enc")
                    nc.gpsimd.dma_start(
                        out=enc_c[:, 0 : csz // 128, :],
                        in_=enc_h[b, pos : pos + csz, :].rearrange(
                            "(j p) e -> p j e", p=128
                        ),
                    )
                    pos += csz
                    chunks.append(enc_c)
                msk_b = msk_pool.tile([1, s], i32, tag="mask")
                nc.sync.dma_start(out=msk_b, in_=msk_h[b : b + 1, :])
                maskf = msk_pool.tile([1, s], bf16, tag="maskf")
                nc.gpsimd.tensor_copy(out=maskf, in_=msk_b)
                state[b] = dict(enc=chunks, pmask=maskf)

        def emit_eproj_scores(b, mid_hook=None):
            chunks = state[b]["enc"]
            scores = sm_pool.tile([1, p_pad], f32, tag="scores")
            pos = 0
            for t, csz in enumerate(chunk_sizes):
                st_c = csz // 128
                if b == 0 and t == 0 and pre_encT is not None:
                    encT = pre_encT
                else:
                    encT = encT_pool.tile([128, n_et, 512], bf16, tag="encT")
                    for j in range(st_c):
                        nc.sync.dma_start(
                            out=encT[:, :, 128 * j : 128 * (j + 1)],
                            in_=chunks[t][:, j, :],
                            transpose=True,
                        )
                psum_sc = sc_psum.tile([1, csz], f32, tag="sc")
                for i in range(n_dt):
                    psum_e = pe_psum.tile([128, csz], f32, tag="pe")
                    for e in range(n_et):
                        nc.tensor.matmul(
                            psum_e,
                            lhsT=w_enc_sb[:, e, 128 * i : 128 * (i + 1)],
                            rhs=encT[:, e, 0:csz],
                            start=(e == 0),
                            stop=(e == n_et - 1),
                        )
                    if mid_hook is not None:
                        # h_proj/hb must be emitted before the first tanh
                        # that reads hb_sb (program-order RAW tracking), but
                        # after d0's matmuls so PE has work while w_dec lands
                        mid_hook()
                        mid_hook = None
                    th = tanh_pool.tile([128, csz], bf16, tag="tanh")
                    nc.scalar.activation(
                        th, psum_e, Tanh, bias=hb_sb[:, i, b : b + 1], scale=1.0
                    )
                    nc.tensor.matmul(
                        psum_sc,
                        lhsT=v_sb[:, i : i + 1],
                        rhs=th,
                        start=(i == 0),
                        stop=(i == n_dt - 1),
                    )
                nc.scalar.copy(scores[:, pos : pos + csz], psum_sc)
                pos += csz
            state[b]["scores"] = scores

        def emit_softmax(b):
            scores = state[b]["scores"]
            pmask = state[b]["pmask"]
            # no max-shift needed: |score| <= sum|v_d| = 32 strictly
            # (|tanh|<=1, |v_w|<=1/32), so exp cannot overflow fp32
            nc.scalar.activation(scores, scores, Exp, bias=0.0, scale=1.0)
            nc.vector.tensor_mul(scores, scores, pmask)
            ssum = small_pool.tile([1, 1], f32, tag="ssum")
            nc.vector.reduce_sum(out=ssum, in_=scores, axis=mybir.AxisListType.X)
            rsum = small_pool.tile([1, 1], f32, tag="rsum")
            nc.vector.reciprocal(rsum, ssum)
            attn_bf = sm_pool.tile([1, p_pad], bf16, tag="attn")
            nc.vector.tensor_scalar_mul(attn_bf, scores, rsum[0:1, 0:1])
            state[b]["attn"] = attn_bf

        def emit_attnT_weighted(b):
            chunks = state[b]["enc"]
            attn_bf = state[b]["attn"]
            # transpose attn row into columns: K=1 matmul against ones(1,1)
            psum_at = at_psum.tile([128, n_ct], f32, tag="at")
            for j in range(n_ct):
                nc.tensor.matmul(
                    psum_at[:, j : j + 1],
                    lhsT=attn_bf[:, 128 * j : 128 * (j + 1)],
                    rhs=ones_bf,
                    start=True,
                    stop=True,
                )
            attnT = small_pool.tile([128, n_ct], bf16, tag="attnT")
            nc.scalar.copy(attnT, psum_at)

            out_sb = outsb_pool.tile([1, e_dim], f32, tag="outsb")
            for ec in range(n_ec):
                psum_w = w_psum.tile([1, 512], f32, tag="w")
                for j in range(n_ct):
                    c, jj = tile_map[j]
                    nc.tensor.matmul(
                        psum_w,
                        lhsT=attnT[:, j : j + 1],
                        rhs=chunks[c][:, jj, 512 * ec : 512 * (ec + 1)],
                        start=(j == 0),
                        stop=(j == n_ct - 1),
                    )
                nc.scalar.copy(out_sb[:, 512 * ec : 512 * (ec + 1)], psum_w)
            nc.sync.dma_start(out=out_h[b : b + 1, :], in_=out_sb)

        # sparse: batch-0's index build + gathers overlap the weight DMA.
        # dense: batch-0 chunk 0 loads first, then w_enc (e_proj's weights),
        # then w_dec — so the first e_proj matmuls start ~12us in and the
        # tiny h_proj fills the remaining DMA latency
        if sparse:
            emit_loads(0)
        w_enc_sb = consts.tile([128, n_et, d_dim], bf16)
        nc.gpsimd.dma_start(
            out=w_enc_sb, in_=aw_h[d_dim:, :].rearrange("(k p) d -> p k d", p=128)
        )
        pre_encT = None
        if not sparse:
            enc_b0_c0 = enc_pool.tile([128, 4, e_dim], bf16, tag="enc")
            nc.gpsimd.dma_start(
                out=enc_b0_c0[:, 0 : chunk_sizes[0] // 128, :],
                in_=enc_h[0, 0 : chunk_sizes[0], :].rearrange(
                    "(j p) e -> p j e", p=128
                ),
            )
        wd_sb = consts.tile([128, n_dt, d_dim], bf16)
        nc.gpsimd.dma_start(
            out=wd_sb, in_=aw_h[0:d_dim, :].rearrange("(k p) d -> p k d", p=128)
        )

        hb_sb = consts.tile([128, n_dt, bc], f32)

        def emit_hproj():
            # hiddenT (d on partitions) via K=bc transpose-by-matmul.
            # PSUM->SBUF copies ride VectorE so they can't head-of-line
            # block the tanh ops already queued on ScalarE.
            psum_h = pe_psum.tile([128, n_dt * bc], f32, tag="pe")
            for k in range(n_dt):
                nc.tensor.matmul(
                    psum_h[:, bc * k : bc * (k + 1)],
                    lhsT=hs_bf[:, 128 * k : 128 * (k + 1)],
                    rhs=ident_sb,
                    start=True,
                    stop=True,
                )
            hT_sb = consts.tile([128, n_dt, bc], bf16)
            nc.vector.tensor_copy(hT_sb, psum_h)

            # h_projT[d, b] accumulated over dec-in tiles. One PSUM group
            # per (k, i) — PSUM start=True arms pending-zero for the whole
            # 2 KiB region, so cross-k accumulation happens in SBUF.
            hacc = consts.tile([128, n_dt * bc], f32)
            for k in range(n_dt):
                psum_hp = pe_psum.tile([128, n_dt * bc], f32, tag="pe")
                for i in range(n_dt):
                    nc.tensor.matmul(
                        psum_hp[:, bc * i : bc * (i + 1)],
                        lhsT=wd_sb[:, k, 128 * i : 128 * (i + 1)],
                        rhs=hT_sb[:, k, :],
                        start=True,
                        stop=True,
                    )
                if k == 0:
                    nc.vector.tensor_copy(hacc, psum_hp)
                else:
                    nc.vector.tensor_add(hacc, hacc, psum_hp)
            # hb[d, b] = h_projT + a_b  (per-partition bias for the tanh)
            for i in range(n_dt):
                nc.vector.tensor_scalar_add(
                    hb_sb[:, i, :], hacc[:, bc * i : bc * (i + 1)], ab_sb[:, i : i + 1]
                )

        if sparse:
            emit_hproj()
        if not sparse:
            emit_loads(0)
        if sparse and bc > 1:
            # two batches of load lookahead: the per-batch index-build +
            # scatter chain is ~Pool-bound and needs a head start
            emit_loads(1)

        # interleave so PE never waits on a softmax: weighted(b-1) runs
        # while softmax(b) is still on VectorE/ScalarE. attnT/weighted are
        # emitted BEFORE softmax(b) so their semaphore waits can't get
        # coarsened into waiting on batch b's softmax ops.
        for b in range(bc):
            if b > 0 and not (sparse and b == 1):
                emit_loads(b)
            emit_eproj_scores(
                b, mid_hook=emit_hproj if (b == 0 and not sparse) else None
            )
            if b >= 1:
                emit_attnT_weighted(b - 1)
            emit_softmax(b)
        emit_attnT_weighted(bc - 1)

    nc.compile()
    return nc


_CACHE = {}


def kernel(hidden_states, encoder_outputs, encoder_masks, a_w, a_b, v_w):
    import ml_dtypes
    from concourse.bass_utils import run_bass_kernel_spmd

    if "nc" not in _CACHE:
        _CACHE["nc"] = build_bass_kernel()
    nc = _CACHE["nc"]

    hidden_states = np.asarray(hidden_states, dtype=np.float32)
    encoder_outputs = np.asarray(encoder_outputs, dtype=np.float32)
    encoder_masks = np.asarray(encoder_masks, dtype=np.int32)
    a_w = np.ascontiguousarray(np.asarray(a_w, dtype=np.float32))
    a_b = np.ascontiguousarray(np.asarray(a_b, dtype=np.float32))
    v_w = np.ascontiguousarray(np.asarray(v_w, dtype=np.float32))
    ident = np.eye(BC, dtype=ml_dtypes.bfloat16)

    in_maps = []
    for c in range(N_CORES):
        sl = slice(c * BC, (c + 1) * BC)
        m = {
            "hidden_states": np.ascontiguousarray(hidden_states[sl]),
            "encoder_outputs": np.ascontiguousarray(encoder_outputs[sl]),
            "encoder_masks": np.ascontiguousarray(encoder_masks[sl]),
            "a_w": a_w,
            "a_b": a_b,
            "v_w": v_w,
            "ident": ident,
        }
        if SPARSE:
            m["iota_pf"] = np.arange(P_PAD, dtype=np.float32).reshape(1, P_PAD)
            tok = (
                np.arange(S // 128)[None, :] * 128 + np.arange(128)[:, None]
            ).astype(np.int32)
            m["tokrep"] = np.repeat(tok[:, :, None], 4, axis=2).copy()
            m["iota_ppi"] = np.ascontiguousarray(tok[:, : P_PAD // 128])
        in_maps.append(m)

    global _LAST_IN_MAPS
    _LAST_IN_MAPS = in_maps
    res = run_bass_kernel_spmd(nc, in_maps, core_ids=list(range(N_CORES)))
    out = np.concatenate([r["out"] for r in res.results], axis=0)
    return out.astype(np.float32)


_LAST_IN_MAPS = None



# revision 10
# speedup vs baseline: 2.4380x; 2.4380x over previous
"""Bahdanau-style attention kernel for Trainium2 (8 NeuronCores, SPMD).

Math (per batch row b):
    h_proj = hidden @ a_w[:DEC]                       (DEC,)
    e_proj[s, :] = enc[s, :] @ a_w[DEC:]              (S, DEC)
    energy = tanh(e_proj + h_proj + a_b)              (S, DEC)
    scores = energy @ v_w                             (S,)
    scores = where(mask == 0, -1e10, scores)
    attn = softmax(scores)                            (S,)
    out = attn @ enc                                  (ENC,)

Sharding: data-parallel over batch (32 rows -> 4 rows on each of 8 cores);
weights replicated.

Per-core pipeline (per 512-token chunk t of batch row b):
  - enc chunk DMA-loaded with fp32->bf16 cast (SWDGE) in natural (s, e)
    layout; kept for the final weighted sum.
  - encT built by PE transpose-mode matmuls (128x128 tiles) into PSUM
    (bf16), evacuated to SBUF with a fused bf16->fp8e4m3 cast
    (ScalarE/DVE/GpSimd split).
  - e_proj computed transposed (d on partitions) with fp8 DoubleRow
    matmuls (K=256 per instruction): lhsT = host-prequantized
    w_enc * 64 in fp8, rhs = encT fp8 pairs.  The 1/64 rescale rides the
    tanh activation's scale; (h_proj + a_b) is its per-partition bias.
  - scores = v . tanh computed as columns: N=1 matmuls with th 128x128
    slices as the stationary operand, v column as moving -> scoresT
    accumulates in a [128, 16] PSUM tile.
  - softmax unnormalized: mask folded in as a (mask-1)*1e10 bias added to
    scoresT PSUM, then Exp on ScalarE with accum_out giving per-partition
    sums; denominator closed by one cross-partition N=1 matmul; the
    1/sum rescale is applied once to the final weighted sum.
  - weighted sum as N=1 matmuls: lhsT = natural-layout enc 128x128
    slices (bf16, unquantized - fp8 here would put ~4% noise on the
    output), rhs = p column.
"""

import numpy as np
from contextlib import ExitStack

B, S, ENC, DEC = 32, 2048, 1024, 1024
N_CORES = 8
BC = B // N_CORES   # batch rows per core
W_SCALE = 64.0      # fp8 weight pre-scale (avoids e4m3 subnormal range)


def build_bass_kernel(bc=BC, s=S, e_dim=ENC, d_dim=DEC, debug=False):
    import concourse.bass as bass
    import concourse.tile as tile
    from concourse import bacc, mybir

    f32 = mybir.dt.float32
    bf16 = mybir.dt.bfloat16
    fp8 = mybir.dt.float8e4
    i32 = mybir.dt.int32
    Tanh = mybir.ActivationFunctionType.Tanh
    Exp = mybir.ActivationFunctionType.Exp
    Alu = mybir.AluOpType
    DR = mybir.MatmulPerfMode.DoubleRow

    assert s % 512 == 0 and e_dim % 128 == 0 and d_dim % 128 == 0
    n_t = s // 512                 # 512-token chunks per batch row
    n_st = 4                       # 128-token s-subtiles per chunk
    n_et = e_dim // 128            # e 128-tiles (contraction for e_proj)
    n_dt = d_dim // 128            # d 128-tiles (e_proj output tiles)
    n_ct = s // 128                # s-tiles per batch row (scoresT cols)
    n_kk = n_et // 2               # DoubleRow K=256 steps

    nc = bacc.Bacc("TRN2", target_bir_lowering=False, debug=debug)

    hs_h = nc.dram_tensor("hidden_states", [bc, d_dim], f32, kind="ExternalInput")
    enc_h = nc.dram_tensor("encoder_outputs", [bc, s, e_dim], f32, kind="ExternalInput")
    msk_h = nc.dram_tensor("encoder_masks", [bc, s], i32, kind="ExternalInput")
    aw_h = nc.dram_tensor("a_w", [e_dim + d_dim, d_dim], f32, kind="ExternalInput")
    ab_h = nc.dram_tensor("a_b", [d_dim], f32, kind="ExternalInput")
    vw_h = nc.dram_tensor("v_w", [d_dim], f32, kind="ExternalInput")
    wenc8_h = nc.dram_tensor("w_enc_fp8", [128, n_et, d_dim], fp8, kind="ExternalInput")
    id_h = nc.dram_tensor("ident", [128, 128], bf16, kind="ExternalInput")
    out_h = nc.dram_tensor("out", [bc, e_dim], f32, kind="ExternalOutput")

    with tile.TileContext(nc) as tc, ExitStack() as ctx:
        consts = ctx.enter_context(tc.tile_pool(name="consts", bufs=1))
        enc_pool = ctx.enter_context(tc.tile_pool(name="enc", bufs=9))
        encT_pool = ctx.enter_context(tc.tile_pool(name="encT", bufs=2))
        th_pool = ctx.enter_context(tc.tile_pool(name="tanh", bufs=17))
        p_pool = ctx.enter_context(tc.tile_pool(name="p", bufs=2))
        msk_pool = ctx.enter_context(tc.tile_pool(name="mask", bufs=4))
        small_pool = ctx.enter_context(tc.tile_pool(name="small", bufs=6))
        outsb_pool = ctx.enter_context(tc.tile_pool(name="outsb", bufs=2))
        pe_psum = ctx.enter_context(tc.tile_pool(name="pe_psum", bufs=2, space="PSUM"))
        tr_psum = ctx.enter_context(tc.tile_pool(name="tr_psum", bufs=2, space="PSUM"))
        sc_psum = ctx.enter_context(tc.tile_pool(name="sc_psum", bufs=2, space="PSUM"))
        w_psum = ctx.enter_context(tc.tile_pool(name="w_psum", bufs=1, space="PSUM"))
        mi_psum = ctx.enter_context(tc.tile_pool(name="mi_psum", bufs=1, space="PSUM"))

        # ---------------- prep: small tensors ----------------
        id_sb = consts.tile([128, 128], bf16)
        nc.sync.dma_start(out=id_sb, in_=id_h[:, :])

        hs_bf = consts.tile([bc, d_dim], bf16)
        nc.gpsimd.dma_start(out=hs_bf, in_=hs_h[:, :])  # cast f32->bf16

        v_sb = consts.tile([128, n_dt], bf16)
        nc.gpsimd.dma_start(out=v_sb, in_=vw_h[:].rearrange("(i p) -> p i", p=128))

        ab_sb = consts.tile([128, n_dt], f32)
        nc.sync.dma_start(out=ab_sb, in_=ab_h[:].rearrange("(i p) -> p i", p=128))

        ones_col = consts.tile([128, 1], f32)
        nc.vector.memset(ones_col, 1.0)
        ones_row = consts.tile([1, 128], f32)
        nc.vector.memset(ones_row, 1.0)

        # first enc chunk early so PE work starts ASAP
        enc_chunks = {}

        def emit_load(b, t):
            enc_c = enc_pool.tile([128, n_st, e_dim], bf16, tag="enc")
            nc.gpsimd.dma_start(
                out=enc_c,
                in_=enc_h[b, 512 * t : 512 * (t + 1), :].rearrange(
                    "(j p) e -> p j e", p=128
                ),
            )
            enc_chunks[(b, t)] = enc_c

        emit_load(0, 0)

        wenc8_sb = consts.tile([128, n_et, d_dim], fp8)
        nc.sync.dma_start(out=wenc8_sb, in_=wenc8_h[:, :, :])

        emit_load(0, 1)

        wd_sb = consts.tile([128, n_dt, d_dim], bf16)
        nc.gpsimd.dma_start(
            out=wd_sb, in_=aw_h[0:d_dim, :].rearrange("(k p) d -> p k d", p=128)
        )

        # per-b mask bias columns: (mask - 1) * 1e10  -> 0 for keep, -1e10 drop
        mbias = {}

        def emit_mask(b):
            mcol = msk_pool.tile([128, n_ct], i32, tag="mcol")
            nc.sync.dma_start(
                out=mcol, in_=msk_h[b, :].rearrange("(j p) -> p j", p=128)
            )
            mb = msk_pool.tile([128, n_ct], f32, tag="mbias")
            nc.vector.tensor_scalar(mb, mcol, -1.0, 1e10, op0=Alu.add, op1=Alu.mult)
            mbias[b] = mb

        # ---------------- h_proj (tiny) ----------------
        hb_sb = consts.tile([128, n_dt, bc], f32)

        def emit_hproj():
            # hiddenT (d on partitions) via K=bc transpose-by-matmul
            psum_h = pe_psum.tile([128, n_dt * bc], f32, tag="pe")
            for k in range(n_dt):
                nc.tensor.matmul(
                    psum_h[:, bc * k : bc * (k + 1)],
                    lhsT=hs_bf[:, 128 * k : 128 * (k + 1)],
                    rhs=id_sb[0:bc, 0:bc],
                    start=True,
                    stop=True,
                )
            hT_sb = consts.tile([128, n_dt, bc], bf16)
            nc.vector.tensor_copy(hT_sb, psum_h)

            hacc = consts.tile([128, n_dt * bc], f32)
            for k in range(n_dt):
                psum_hp = pe_psum.tile([128, n_dt * bc], f32, tag="pe")
                for i in range(n_dt):
                    nc.tensor.matmul(
                        psum_hp[:, bc * i : bc * (i + 1)],
                        lhsT=wd_sb[:, k, 128 * i : 128 * (i + 1)],
                        rhs=hT_sb[:, k, :],
                        start=True,
                        stop=True,
                    )
                if k == 0:
                    nc.vector.tensor_copy(hacc, psum_hp)
                else:
                    nc.vector.tensor_add(hacc, hacc, psum_hp)
            for i in range(n_dt):
                nc.vector.tensor_scalar_add(
                    hb_sb[:, i, :], hacc[:, bc * i : bc * (i + 1)], ab_sb[:, i : i + 1]
                )

        # ---------------- per-chunk stages ----------------
        state = {}

        def emit_transposes(b, t):
            """encT (fp8, DoubleRow-paired) for chunk (b, t) via PE transpose
            + cast-evacuation. Evac engines: ScalarE for kk==0, DVE else
            (GpSimd has no PSUM port)."""
            chunk = enc_chunks[(b, t)]
            encT8 = encT_pool.tile([128, n_et, 512], fp8, tag="encT")
            for kk in range(n_kk):
                tp = tr_psum.tile([128, 2, n_st, 128], bf16, tag="tr")
                for half in range(2):
                    et = 2 * kk + half
                    for j in range(n_st):
                        nc.tensor.transpose(
                            tp[:, half, j, :],
                            chunk[:, j, 128 * et : 128 * (et + 1)],
                            id_sb,
                        )
                src = tp.rearrange("p a b c -> p a (b c)")
                dst = encT8[:, 2 * kk : 2 * kk + 2, :]
                if kk == 0:
                    nc.scalar.copy(dst, src)
                else:
                    nc.vector.tensor_copy(dst, src)
            state[(b, t)] = encT8

        def emit_eproj(b, t):
            encT8 = state.pop((b, t))
            if t == 0:
                state[("sc", b)] = sc_psum.tile(
                    [128, n_ct], f32, tag="sc", name="psc"
                )
            ths = []
            for i in range(n_dt):
                pe = pe_psum.tile([128, 512], f32, tag="pe")
                for kk in range(n_kk):
                    nc.tensor.matmul(
                        pe,
                        lhsT=wenc8_sb[:, 2 * kk : 2 * kk + 2, 128 * i : 128 * (i + 1)],
                        rhs=encT8[:, 2 * kk : 2 * kk + 2, :],
                        start=(kk == 0),
                        stop=(kk == n_kk - 1),
                        perf_mode=DR,
                    )
                th = th_pool.tile([128, 512], bf16, tag="tanh")
                nc.scalar.activation(
                    th, pe, Tanh, bias=hb_sb[:, i, b : b + 1], scale=1.0 / W_SCALE
                )
                ths.append(th)
            state[("th", b, t)] = ths

        def emit_scores(b, t):
            # Column-outer, i-inner: accumulation groups in the scoresT bank
            # must be strictly sequential (a group's start=True clears
            # has_written for the WHOLE bank).
            ths = state.pop(("th", b, t))
            psum_sc = state[("sc", b)]
            for j in range(n_st):
                col = n_st * t + j
                for i in range(n_dt):
                    nc.tensor.matmul(
                        psum_sc[:, col : col + 1],
                        lhsT=ths[i][:, 128 * j : 128 * (j + 1)],
                        rhs=v_sb[:, i : i + 1],
                        start=(i == 0),
                        stop=(i == n_dt - 1),
                    )

        def emit_softmax_a(b):
            """Mask-bias + exp with fused row-sums (DVE + ScalarE only)."""
            psum_sc = state.pop(("sc", b))
            nc.vector.tensor_add(psum_sc, psum_sc, mbias.pop(b))
            p_bf = p_pool.tile([128, n_ct], bf16, tag="p")
            rowsum = small_pool.tile([128, 1], f32, tag="rowsum")
            nc.scalar.activation(
                p_bf, psum_sc, Exp, bias=0.0, scale=1.0, accum_out=rowsum
            )
            state[("p", b)] = p_bf
            state[("rowsum", b)] = rowsum

        def emit_ssum_recip(b):
            """Cross-partition denominator + reciprocal, a stage after
            softmax_a so the PE/DVE queue heads never block on them."""
            rowsum = state.pop(("rowsum", b))
            ssum = mi_psum.tile([1, 1], f32, tag="mi")
            nc.tensor.matmul(ssum, lhsT=rowsum, rhs=ones_col, start=True, stop=True)
            rsum = small_pool.tile([1, 1], f32, tag="rsum")
            nc.vector.reciprocal(rsum, ssum)
            state[("rsum", b)] = rsum

        def emit_weighted(b):
            p_bf = state.pop(("p", b))
            rsum = state.pop(("rsum", b))
            rbc_ps = mi_psum.tile([128, 1], f32, tag="mi")
            nc.tensor.matmul(rbc_ps, lhsT=ones_row, rhs=rsum, start=True, stop=True)
            rbc = small_pool.tile([128, 1], f32, tag="rbc")
            nc.vector.tensor_copy(rbc, rbc_ps)
            w_ps = w_psum.tile([128, n_dt], f32, tag="w")
            for i in range(n_et):
                for c in range(n_ct):
                    t, j = divmod(c, n_st)
                    nc.tensor.matmul(
                        w_ps[:, i : i + 1],
                        lhsT=enc_chunks[(b, t)][:, j, 128 * i : 128 * (i + 1)],
                        rhs=p_bf[:, c : c + 1],
                        start=(c == 0),
                        stop=(c == n_ct - 1),
                    )
            for t in range(n_t):
                del enc_chunks[(b, t)]
            out_sb = outsb_pool.tile([128, n_et], f32, tag="outsb")
            nc.vector.tensor_scalar_mul(out_sb, w_ps, rbc[:, 0:1])
            nc.sync.dma_start(
                out=out_h[b, :].rearrange("(i p) -> p i", p=128), in_=out_sb
            )

        # ---------------- schedule ----------------
        # Software-pipelined: at stage idx the PE emission order is
        #   transposes(idx+1) -> e_proj(idx) -> scores(idx-1)
        # so the (in-order) PE queue always has transpose/e_proj work while
        # the cast-evacuation of the next chunk and the tanh of the current
        # one complete on ScalarE/DVE.
        stages = [(b, t) for b in range(bc) for t in range(n_t)]
        emit_mask(0)
        emit_hproj()
        emit_transposes(0, 0)
        for idx, (b, t) in enumerate(stages):
            if idx + 2 < len(stages):
                emit_load(*stages[idx + 2])
            if idx + 1 < len(stages):
                nb, nt = stages[idx + 1]
                if nt == 0:
                    emit_mask(nb)
                emit_transposes(nb, nt)
            emit_eproj(b, t)
            if idx >= 1:
                pb, pt = stages[idx - 1]
                emit_scores(pb, pt)
                if pt == n_t - 1:
                    emit_softmax_a(pb)
            if b > 0:
                if t == 1:
                    emit_ssum_recip(b - 1)
                elif t == 2:
                    emit_weighted(b - 1)
        emit_scores(*stages[-1])
        emit_softmax_a(bc - 1)
        emit_ssum_recip(bc - 1)
        emit_weighted(bc - 1)

    nc.compile()
    return nc


_CACHE = {}


def _prep_weights(a_w):
    """Host-side weight repack: w_enc * 64 quantized to fp8e4m3 in
    (p, k, d) layout matching the DoubleRow stationary-operand slices."""
    import ml_dtypes

    w_enc = np.asarray(a_w[DEC:], dtype=np.float32)  # (ENC, DEC)
    w = (w_enc * W_SCALE).reshape(ENC // 128, 128, DEC).transpose(1, 0, 2)
    return np.ascontiguousarray(w).astype(ml_dtypes.float8_e4m3)


def kernel(hidden_states, encoder_outputs, encoder_masks, a_w, a_b, v_w):
    import ml_dtypes
    from concourse.bass_utils import run_bass_kernel_spmd

    if "nc" not in _CACHE:
        _CACHE["nc"] = build_bass_kernel()
    nc = _CACHE["nc"]

    hidden_states = np.asarray(hidden_states, dtype=np.float32)
    encoder_outputs = np.asarray(encoder_outputs, dtype=np.float32)
    encoder_masks = np.asarray(encoder_masks, dtype=np.int32)
    a_w = np.ascontiguousarray(np.asarray(a_w, dtype=np.float32))
    a_b = np.ascontiguousarray(np.asarray(a_b, dtype=np.float32))
    v_w = np.ascontiguousarray(np.asarray(v_w, dtype=np.float32))
    ident = np.eye(128, dtype=ml_dtypes.bfloat16)
    wenc8 = _prep_weights(a_w)

    in_maps = []
    for c in range(N_CORES):
        sl = slice(c * BC, (c + 1) * BC)
        in_maps.append(
            {
                "hidden_states": np.ascontiguousarray(hidden_states[sl]),
                "encoder_outputs": np.ascontiguousarray(encoder_outputs[sl]),
                "encoder_masks": np.ascontiguousarray(encoder_masks[sl]),
                "a_w": a_w,
                "a_b": a_b,
                "v_w": v_w,
                "w_enc_fp8": wenc8,
                "ident": ident,
            }
        )

    global _LAST_IN_MAPS
    _LAST_IN_MAPS = in_maps
    res = run_bass_kernel_spmd(nc, in_maps, core_ids=list(range(N_CORES)))
    out = np.concatenate([r["out"] for r in res.results], axis=0)
    return out.astype(np.float32)


_LAST_IN_MAPS = None


# revision 17
# speedup vs baseline: 3.5417x; 1.4527x over previous
"""Bahdanau-style attention kernel for Trainium2 (8 NeuronCores, SPMD).

Math (per batch row b):
    h_proj = hidden @ a_w[:DEC]                       (DEC,)
    e_proj[s, :] = enc[s, :] @ a_w[DEC:]              (S, DEC)
    energy = tanh(e_proj + h_proj + a_b)              (S, DEC)
    scores = energy @ v_w                             (S,)
    scores = where(mask == 0, -1e10, scores)
    attn = softmax(scores)                            (S,)
    out = attn @ enc                                  (ENC,)

Sharding: data-parallel over batch (32 rows -> 4 rows on each of 8 cores);
weights replicated.

Masked tokens get attn == 0 exactly, so only the unmasked rows (~half;
Binomial(2048, .5), padded to P_PAD=1152 = +5.7 sigma) contribute to any
output. The host computes each row's unmasked-index list (cheap metadata,
<0.01% of the FLOPs - the kernel-side equivalent was measured
descriptor-bound on Q7) and the device gathers just those encoder rows
with indirect SWDGE DMAs (fp32->bf16 cast in flight, one 128-index call
per tile - the silicon-validated gather shape). Pad lanes are killed by
a host-built -1e10 compact-mask bias, so the math is exactly the
reference's masked softmax.

Per-core pipeline per batch row (9 compact 128-token tiles as chunks of
512/512/128):
  - encT built by PE transpose-mode matmuls (128x128 tiles) into PSUM
    (bf16), evacuated to SBUF with a fused bf16->fp8e4m3 cast (split
    ScalarE/DVE; GpSimd has no PSUM port).
  - e_proj transposed (d on partitions) with fp8 DoubleRow matmuls
    (K=256 per instruction): lhsT = host-prequantized w_enc * 64 fp8,
    rhs = encT fp8 pairs. The 1/64 rescale and (h_proj + a_b) ride the
    tanh activation's scale/bias; tanh runs on [128, 1024] tiles (chunk
    pairs) to halve the per-op ScalarE init cost.
  - scores = v . tanh as columns: N=1 matmuls, th 128x128 slices
    stationary, v column moving -> scoresT in a [128, 9] PSUM tile
    (accumulation groups strictly sequential per column - start=True
    clears has_written bank-wide).
  - softmax unnormalized: compact-mask bias added to scoresT PSUM, Exp
    on ScalarE with accum_out row-sums, denominator closed by one
    cross-partition N=1 matmul; the 1/sum rescale lands once on the
    final weighted sum.
  - weighted sum as N=1 matmuls: lhsT = natural-layout gathered rows
    (bf16, unquantized - fp8 enc here would put ~4% noise on the
    output), rhs = p column.
"""

import numpy as np
from contextlib import ExitStack

B, S, ENC, DEC = 32, 2048, 1024, 1024
N_CORES = 8
BC = B // N_CORES   # batch rows per core
W_SCALE = 64.0      # fp8 weight pre-scale (avoids e4m3 subnormal range)
# padded compact-token count: Binomial(2048, 0.5) is 1024 +- 22.6, so 1152
# is a +5.7 sigma bound (seed-0 data maxes at 1062)
P_PAD = 1152
CHUNKS = (512, 512, 128)   # compact tokens per chunk (= 9 tiles of 128)


def build_bass_kernel(bc=BC, s=S, e_dim=ENC, d_dim=DEC, debug=False):
    import concourse.bass as bass
    import concourse.tile as tile
    from concourse import bacc, mybir

    f32 = mybir.dt.float32
    bf16 = mybir.dt.bfloat16
    fp8 = mybir.dt.float8e4
    i32 = mybir.dt.int32
    Tanh = mybir.ActivationFunctionType.Tanh
    Exp = mybir.ActivationFunctionType.Exp
    DR = mybir.MatmulPerfMode.DoubleRow

    n_et = e_dim // 128            # e 128-tiles (contraction for e_proj)
    n_dt = d_dim // 128            # d 128-tiles (e_proj output tiles)
    n_gt = P_PAD // 128            # compact s-tiles per batch row (9)
    n_kk = n_et // 2               # DoubleRow K=256 steps
    # (chunk, tile-within-chunk) for each global compact tile
    tile_map = []
    for c, csz in enumerate(CHUNKS):
        for jj in range(csz // 128):
            tile_map.append((c, jj))

    nc = bacc.Bacc("TRN2", target_bir_lowering=False, debug=debug)

    hs_h = nc.dram_tensor("hidden_states", [bc, d_dim], f32, kind="ExternalInput")
    enc_h = nc.dram_tensor("encoder_outputs", [bc, s, e_dim], f32, kind="ExternalInput")
    gidx_h = nc.dram_tensor("gidx", [bc, 128, n_gt], i32, kind="ExternalInput")
    cbias_h = nc.dram_tensor("cbias", [bc, 128, n_gt], f32, kind="ExternalInput")
    ab_h = nc.dram_tensor("a_b", [d_dim], f32, kind="ExternalInput")
    vw_h = nc.dram_tensor("v_w", [d_dim], f32, kind="ExternalInput")
    wenc8_h = nc.dram_tensor("w_enc_fp8", [128, n_et, d_dim], fp8, kind="ExternalInput")
    wd8_h = nc.dram_tensor("w_dec_fp8", [128, n_dt, d_dim], fp8, kind="ExternalInput")
    id_h = nc.dram_tensor("ident", [128, 128], bf16, kind="ExternalInput")
    out_h = nc.dram_tensor("out", [bc, e_dim], f32, kind="ExternalOutput")

    enc_flat = enc_h[:, :, :].rearrange("b s e -> (b s) e")

    with tile.TileContext(nc) as tc, ExitStack() as ctx:
        consts = ctx.enter_context(tc.tile_pool(name="consts", bufs=1))
        enc_pool = ctx.enter_context(tc.tile_pool(name="enc", bufs=10))
        encT_pool = ctx.enter_context(tc.tile_pool(name="encT", bufs=4))
        th_pool = ctx.enter_context(tc.tile_pool(name="tanh", bufs=10))
        p_pool = ctx.enter_context(tc.tile_pool(name="p", bufs=2))
        small_pool = ctx.enter_context(tc.tile_pool(name="small", bufs=6))
        outsb_pool = ctx.enter_context(tc.tile_pool(name="outsb", bufs=2))
        pe_psum = ctx.enter_context(tc.tile_pool(name="pe_psum", bufs=2, space="PSUM"))
        tr_psum = ctx.enter_context(tc.tile_pool(name="tr_psum", bufs=2, space="PSUM"))
        sc_psum = ctx.enter_context(tc.tile_pool(name="sc_psum", bufs=1, space="PSUM"))
        w_psum = ctx.enter_context(tc.tile_pool(name="w_psum", bufs=1, space="PSUM"))

        # ---------------- prologue DMAs (transfers serialize; this order
        # is the pipeline-fill critical path) ----------------
        id_sb = consts.tile([128, 128], bf16)
        nc.sync.dma_start(out=id_sb, in_=id_h[:, :])

        hs_bf = consts.tile([bc, d_dim], bf16)
        nc.gpsimd.dma_start(out=hs_bf, in_=hs_h[:, :])  # cast f32->bf16

        gidx_sb = consts.tile([128, bc, n_gt], i32)
        nc.sync.dma_start(out=gidx_sb, in_=gidx_h[:, :, :].rearrange("b p g -> p b g"))

        cbias_sb = consts.tile([128, bc, n_gt], f32)
        nc.sync.dma_start(
            out=cbias_sb, in_=cbias_h[:, :, :].rearrange("b p g -> p b g")
        )

        enc_chunks = {}

        def emit_gather(b, c):
            """Gather unmasked encoder rows for chunk (b, c): one
            128-index SWDGE call per 128-token tile, f32->bf16 cast in
            the DMA."""
            enc_c = enc_pool.tile([128, 4, e_dim], bf16, tag="enc")
            g0 = sum(cs // 128 for cs in CHUNKS[:c])
            for jj in range(CHUNKS[c] // 128):
                nc.gpsimd.indirect_dma_start(
                    out=enc_c[:, jj, :],
                    out_offset=None,
                    in_=enc_flat,
                    in_offset=bass.IndirectOffsetOnAxis(
                        ap=gidx_sb[:, b, g0 + jj : g0 + jj + 1], axis=0
                    ),
                )
            enc_chunks[(b, c)] = enc_c

        emit_gather(0, 0)
        emit_gather(0, 1)
        emit_gather(0, 2)

        wenc8_sb = consts.tile([128, n_et, d_dim], fp8)
        nc.sync.dma_start(out=wenc8_sb, in_=wenc8_h[:, :, :])

        wd8_sb = consts.tile([128, n_dt, d_dim], fp8)
        nc.sync.dma_start(out=wd8_sb, in_=wd8_h[:, :, :])

        emit_gather(1, 0)
        emit_gather(1, 1)
        emit_gather(1, 2)

        v_sb = consts.tile([128, n_dt], bf16)
        nc.gpsimd.dma_start(out=v_sb, in_=vw_h[:].rearrange("(i p) -> p i", p=128))

        ab_sb = consts.tile([128, n_dt], f32)
        nc.sync.dma_start(out=ab_sb, in_=ab_h[:].rearrange("(i p) -> p i", p=128))

        ones_col = consts.tile([128, 1], f32)
        nc.vector.memset(ones_col, 1.0)
        ones_row = consts.tile([1, 128], f32)
        nc.vector.memset(ones_row, 1.0)

        # ---------------- h_proj (tiny; emitted via mid-hook inside the
        # first e_proj so the in-order PE queue isn't head-blocked while
        # w_dec_fp8 is still in flight) ----------------
        hb_sb = consts.tile([128, n_dt, bc], f32)

        def emit_hproj():
            # hiddenT (d on partitions) via K=bc transpose-by-matmul.
            # PSUM comes from tr_psum: pe_psum buffers hold un-evacuated
            # e_proj output whose tanh waits on hb -> using them here
            # would deadlock the PE queue.
            psum_h = tr_psum.tile([128, n_dt * bc], f32, tag="tr")
            for k in range(n_dt):
                nc.tensor.matmul(
                    psum_h[:, bc * k : bc * (k + 1)],
                    lhsT=hs_bf[:, 128 * k : 128 * (k + 1)],
                    rhs=id_sb[0:bc, 0:bc],
                    start=True,
                    stop=True,
                )
            hT8 = consts.tile([128, n_dt, bc], fp8)
            nc.vector.tensor_copy(hT8, psum_h)

            hacc = consts.tile([128, n_dt * bc], f32)
            for k in range(n_dt):
                psum_hp = tr_psum.tile([128, n_dt * bc], f32, tag="tr")
                for i in range(n_dt):
                    nc.tensor.matmul(
                        psum_hp[:, bc * i : bc * (i + 1)],
                        lhsT=wd8_sb[:, k, 128 * i : 128 * (i + 1)],
                        rhs=hT8[:, k, :],
                        start=True,
                        stop=True,
                    )
                if k == 0:
                    nc.vector.tensor_copy(hacc, psum_hp)
                else:
                    nc.vector.tensor_add(hacc, hacc, psum_hp)
            for i in range(n_dt):
                # hb = hacc / W_SCALE + a_b (w_dec was pre-scaled *64)
                nc.vector.tensor_scalar(
                    hb_sb[:, i, :],
                    hacc[:, bc * i : bc * (i + 1)],
                    1.0 / W_SCALE,
                    ab_sb[:, i : i + 1],
                    op0=mybir.AluOpType.mult,
                    op1=mybir.AluOpType.add,
                )

        # ---------------- per-chunk stages ----------------
        state = {}

        def emit_transposes(b, c):
            """encT (fp8, DoubleRow-paired) for chunk (b, c) via PE
            transpose + cast-evacuation (ScalarE for kk==0 of even
            chunks, DVE otherwise; GpSimd has no PSUM port)."""
            chunk = enc_chunks[(b, c)]
            n_j = CHUNKS[c] // 128
            encT8 = encT_pool.tile([128, n_et, 512], fp8, tag="encT")
            for kk in range(n_kk):
                tp = tr_psum.tile([128, 2, 4, 128], bf16, tag="tr")
                for half in range(2):
                    et = 2 * kk + half
                    for j in range(n_j):
                        nc.tensor.transpose(
                            tp[:, half, j, :],
                            chunk[:, j, 128 * et : 128 * (et + 1)],
                            id_sb,
                        )
                src = tp[:, :, 0:n_j, :].rearrange("p a b c -> p a (b c)")
                dst = encT8[:, 2 * kk : 2 * kk + 2, 0 : 128 * n_j]
                if kk == 0 and c == 0:
                    nc.scalar.copy(dst, src)
                else:
                    nc.vector.tensor_copy(dst, src)
            state[(b, c)] = encT8

        def emit_eproj_pair(b, mid_hook=None):
            # chunks 0+1 together: tanh runs on [128, 1024] tiles (one
            # ScalarE init per two chunks); the two 512-wide matmul
            # groups land in the two banks of a 2-bank PSUM tile.
            eTa = state.pop((b, 0))
            eTb = state.pop((b, 1))
            state[("sc", b)] = sc_psum.tile([128, n_gt], f32, tag="sc", name="psc")
            ths = []
            for i in range(n_dt):
                pe = pe_psum.tile([128, 2, 512], f32, tag="pe")
                for half, eT in ((0, eTa), (1, eTb)):
                    for kk in range(n_kk):
                        nc.tensor.matmul(
                            pe[:, half, :],
                            lhsT=wenc8_sb[
                                :, 2 * kk : 2 * kk + 2, 128 * i : 128 * (i + 1)
                            ],
                            rhs=eT[:, 2 * kk : 2 * kk + 2, :],
                            start=(kk == 0),
                            stop=(kk == n_kk - 1),
                            perf_mode=DR,
                        )
                if mid_hook is not None and i == 1:
                    mid_hook()
                    mid_hook = None
                th = th_pool.tile([128, 2, 512], bf16, tag="tanh")
                nc.scalar.activation(
                    th.rearrange("p a b -> p (a b)"),
                    pe.rearrange("p a b -> p (a b)"),
                    Tanh,
                    bias=hb_sb[:, i, b : b + 1],
                    scale=1.0 / W_SCALE,
                )
                ths.append(th)
            state[("th", b)] = ths

        def emit_eproj_tail(b):
            # chunk 2: single 128-token tile
            eT = state.pop((b, 2))
            ths = []
            for i in range(n_dt):
                pe = pe_psum.tile([128, 2, 512], f32, tag="pe")
                for kk in range(n_kk):
                    nc.tensor.matmul(
                        pe[:, 0, 0:128],
                        lhsT=wenc8_sb[:, 2 * kk : 2 * kk + 2, 128 * i : 128 * (i + 1)],
                        rhs=eT[:, 2 * kk : 2 * kk + 2, 0:128],
                        start=(kk == 0),
                        stop=(kk == n_kk - 1),
                        perf_mode=DR,
                    )
                th = th_pool.tile([128, 128], bf16, tag="ttail")
                nc.scalar.activation(
                    th, pe[:, 0, 0:128], Tanh, bias=hb_sb[:, i, b : b + 1],
                    scale=1.0 / W_SCALE,
                )
                ths.append(th)
            state[("tht", b)] = ths

        def emit_scores_pair(b):
            # Column-outer, i-inner: accumulation groups in the scoresT
            # bank must be strictly sequential (start=True clears
            # has_written for the WHOLE bank).
            ths = state.pop(("th", b))
            psum_sc = state[("sc", b)]
            for col in range(8):
                half, jj = divmod(col, 4)
                for i in range(n_dt):
                    nc.tensor.matmul(
                        psum_sc[:, col : col + 1],
                        lhsT=ths[i][:, half, 128 * jj : 128 * (jj + 1)],
                        rhs=v_sb[:, i : i + 1],
                        start=(i == 0),
                        stop=(i == n_dt - 1),
                    )

        def emit_scores_tail(b):
            ths = state.pop(("tht", b))
            psum_sc = state[("sc", b)]
            for i in range(n_dt):
                nc.tensor.matmul(
                    psum_sc[:, 8:9],
                    lhsT=ths[i],
                    rhs=v_sb[:, i : i + 1],
                    start=(i == 0),
                    stop=(i == n_dt - 1),
                )

        def emit_softmax_a(b):
            """Compact-mask bias + exp with fused row-sums (DVE+ScalarE)."""
            psum_sc = state.pop(("sc", b))
            nc.vector.tensor_add(psum_sc, psum_sc, cbias_sb[:, b, :])
            p_bf = p_pool.tile([128, n_gt], bf16, tag="p")
            rowsum = small_pool.tile([128, 1], f32, tag="rowsum")
            nc.scalar.activation(
                p_bf, psum_sc, Exp, bias=0.0, scale=1.0, accum_out=rowsum
            )
            state[("p", b)] = p_bf
            state[("rowsum", b)] = rowsum

        def emit_ssum_recip(b):
            rowsum = state.pop(("rowsum", b))
            ssum = w_psum.tile([1, 1], f32, tag="w")
            nc.tensor.matmul(ssum, lhsT=rowsum, rhs=ones_col, start=True, stop=True)
            rsum = small_pool.tile([1, 1], f32, tag="rsum")
            nc.vector.reciprocal(rsum, ssum)
            state[("rsum", b)] = rsum

        def emit_weighted(b):
            p_bf = state.pop(("p", b))
            rsum = state.pop(("rsum", b))
            rbc_ps = w_psum.tile([128, 1], f32, tag="w")
            nc.tensor.matmul(rbc_ps, lhsT=ones_row, rhs=rsum, start=True, stop=True)
            rbc = small_pool.tile([128, 1], f32, tag="rbc")
            nc.vector.tensor_copy(rbc, rbc_ps)
            w_ps = w_psum.tile([128, n_dt], f32, tag="w")
            for i in range(n_et):
                for g, (c, jj) in enumerate(tile_map):
                    nc.tensor.matmul(
                        w_ps[:, i : i + 1],
                        lhsT=enc_chunks[(b, c)][:, jj, 128 * i : 128 * (i + 1)],
                        rhs=p_bf[:, g : g + 1],
                        start=(g == 0),
                        stop=(g == n_gt - 1),
                    )
            for c in range(len(CHUNKS)):
                del enc_chunks[(b, c)]
            out_sb = outsb_pool.tile([128, n_et], f32, tag="outsb")
            nc.vector.tensor_scalar_mul(out_sb, w_ps, rbc[:, 0:1])
            nc.sync.dma_start(
                out=out_h[b, :].rearrange("(i p) -> p i", p=128), in_=out_sb
            )

        # ---------------- schedule ----------------
        # Two sub-stages per batch row: A(b) = chunks 0+1 e_proj, B(b) =
        # tail e_proj. Transposes run one sub-stage ahead of their
        # e_proj, scores one sub-stage behind, so the in-order PE queue
        # never blocks on ScalarE/DVE results.
        emit_transposes(0, 0)
        emit_transposes(0, 1)
        for b in range(bc):
            # --- sub-stage A(b) ---
            if b + 2 < bc:
                emit_gather(b + 2, 0)
                emit_gather(b + 2, 1)
            emit_eproj_pair(b, mid_hook=emit_hproj if b == 0 else None)
            emit_transposes(b, 2)
            if b > 0:
                emit_scores_tail(b - 1)
                emit_softmax_a(b - 1)
            # --- sub-stage B(b) ---
            if b + 2 < bc:
                emit_gather(b + 2, 2)
            if b + 1 < bc:
                emit_transposes(b + 1, 0)
                emit_transposes(b + 1, 1)
            emit_eproj_tail(b)
            emit_scores_pair(b)
            if b > 0:
                emit_ssum_recip(b - 1)
                emit_weighted(b - 1)
        emit_scores_tail(bc - 1)
        emit_softmax_a(bc - 1)
        emit_ssum_recip(bc - 1)
        emit_weighted(bc - 1)

    nc.compile()
    return nc


_CACHE = {}


def _prep_weights(a_w):
    """Host-side weight repack: w_enc and w_dec scaled by 64 and
    quantized to fp8e4m3 in (p, k, d) layout matching the stationary-
    operand slices (DoubleRow pairs for w_enc)."""
    import ml_dtypes

    def pack(w):
        w = (np.asarray(w, dtype=np.float32) * W_SCALE).reshape(-1, 128, DEC)
        return np.ascontiguousarray(w.transpose(1, 0, 2)).astype(
            ml_dtypes.float8_e4m3
        )

    return pack(a_w[DEC:]), pack(a_w[:DEC])


def _prep_indices(masks):
    """Per-row unmasked token indices (padded to P_PAD with row 0 of the
    same batch row - its lanes are killed by cbias) and the compact-mask
    bias, both in column-major (p, g) tile layout."""
    bc = masks.shape[0]
    gidx = np.zeros((bc, P_PAD), dtype=np.int32)
    cbias = np.full((bc, P_PAD), -1e10, dtype=np.float32)
    for b in range(bc):
        idx = np.nonzero(masks[b])[0].astype(np.int32)
        cnt = len(idx)
        assert cnt <= P_PAD, f"unmasked count {cnt} exceeds P_PAD={P_PAD}"
        gidx[b, :cnt] = b * S + idx
        gidx[b, cnt:] = b * S
        cbias[b, :cnt] = 0.0
    # (b, tile*128 + p) -> (b, p, tile)
    gidx = np.ascontiguousarray(gidx.reshape(bc, P_PAD // 128, 128).transpose(0, 2, 1))
    cbias = np.ascontiguousarray(
        cbias.reshape(bc, P_PAD // 128, 128).transpose(0, 2, 1)
    )
    return gidx, cbias


def kernel(hidden_states, encoder_outputs, encoder_masks, a_w, a_b, v_w):
    import ml_dtypes
    from concourse.bass_utils import run_bass_kernel_spmd

    if "nc" not in _CACHE:
        _CACHE["nc"] = build_bass_kernel()
    nc = _CACHE["nc"]

    hidden_states = np.asarray(hidden_states, dtype=np.float32)
    encoder_outputs = np.asarray(encoder_outputs, dtype=np.float32)
    encoder_masks = np.asarray(encoder_masks, dtype=np.int32)
    a_w = np.ascontiguousarray(np.asarray(a_w, dtype=np.float32))
    a_b = np.ascontiguousarray(np.asarray(a_b, dtype=np.float32))
    v_w = np.ascontiguousarray(np.asarray(v_w, dtype=np.float32))
    ident = np.eye(128, dtype=ml_dtypes.bfloat16)
    wenc8, wd8 = _prep_weights(a_w)

    in_maps = []
    for c in range(N_CORES):
        sl = slice(c * BC, (c + 1) * BC)
        gidx, cbias = _prep_indices(encoder_masks[sl])
        in_maps.append(
            {
                "hidden_states": np.ascontiguousarray(hidden_states[sl]),
                "encoder_outputs": np.ascontiguousarray(encoder_outputs[sl]),
                "gidx": gidx,
                "cbias": cbias,
                "a_b": a_b,
                "v_w": v_w,
                "w_enc_fp8": wenc8,
                "w_dec_fp8": wd8,
                "ident": ident,
            }
        )

    global _LAST_IN_MAPS
    _LAST_IN_MAPS = in_maps
    res = run_bass_kernel_spmd(nc, in_maps, core_ids=list(range(N_CORES)))
    out = np.concatenate([r["out"] for r in res.results], axis=0)
    return out.astype(np.float32)


_LAST_IN_MAPS = None


# revision 29
# speedup vs baseline: 3.6378x; 1.0271x over previous
"""Bahdanau-style attention kernel for Trainium2 (8 NeuronCores, SPMD).

Math (per batch row b):
    h_proj = hidden @ a_w[:DEC]                       (DEC,)
    e_proj[s, :] = enc[s, :] @ a_w[DEC:]              (S, DEC)
    energy = tanh(e_proj + h_proj + a_b)              (S, DEC)
    scores = energy @ v_w                             (S,)
    scores = where(mask == 0, -1e10, scores)
    attn = softmax(scores)                            (S,)
    out = attn @ enc                                  (ENC,)

Sharding: data-parallel over batch (32 rows -> 4 rows on each of 8 cores);
weights replicated.

Masked tokens get attn == 0 exactly, so only the unmasked rows (~half;
Binomial(2048, .5), padded to P_PAD=1152 = +5.7 sigma) contribute to any
output. The host computes each row's unmasked-index list (cheap metadata,
<0.01% of the FLOPs - the kernel-side equivalent was measured
descriptor-bound on Q7) and the device gathers just those encoder rows
with indirect SWDGE DMAs (fp32->bf16 cast in flight, one 128-index call
per tile - the silicon-validated gather shape). Pad lanes are killed by
a host-built -1e10 compact-mask bias, so the math is exactly the
reference's masked softmax.

Per-core pipeline per batch row (9 compact 128-token tiles as chunks of
512/512/128):
  - encT built by PE transpose-mode matmuls (128x128 tiles) into PSUM
    (bf16), evacuated to SBUF with a fused bf16->fp8e4m3 cast on DVE
    (ScalarE fp8 casts measured noisier on silicon; GpSimd has no PSUM
    port).
  - e_proj transposed (d on partitions) with fp8 DoubleRow matmuls
    (K=256 per instruction): lhsT = host-prequantized w_enc * 64 fp8,
    rhs = encT fp8 pairs. The 1/64 rescale and (h_proj + a_b) ride the
    tanh activation's scale/bias; tanh runs on [128, 1024] tiles (chunk
    pairs) to halve the per-op ScalarE init cost.
  - scores = v . tanh as columns: N=1 matmuls, th 128x128 slices
    stationary, v column moving -> scoresT in a [128, 9] PSUM tile
    (accumulation groups strictly sequential per column - start=True
    clears has_written bank-wide).
  - softmax unnormalized: compact-mask bias added to scoresT PSUM, Exp
    on ScalarE with accum_out row-sums, denominator closed by one
    cross-partition N=1 matmul; the 1/sum rescale lands once on the
    final weighted sum.
  - weighted sum as N=1 matmuls: lhsT = natural-layout gathered rows
    (bf16, unquantized - fp8 enc here would put ~4% noise on the
    output), rhs = p column.
"""

import numpy as np
from contextlib import ExitStack

B, S, ENC, DEC = 32, 2048, 1024, 1024
N_CORES = 8
BC = B // N_CORES   # batch rows per core
W_SCALE = 64.0      # fp8 weight pre-scale (avoids e4m3 subnormal range)
# padded compact-token count: Binomial(2048, 0.5) is 1024 +- 22.6, so 1152
# is a +5.7 sigma bound (seed-0 data maxes at 1062)
P_PAD = 1152
CHUNKS = (512, 512, 128)   # compact tokens per chunk (= 9 tiles of 128)


def build_bass_kernel(bc=BC, s=S, e_dim=ENC, d_dim=DEC, debug=False):
    import concourse.bass as bass
    import concourse.tile as tile
    from concourse import bacc, mybir

    f32 = mybir.dt.float32
    bf16 = mybir.dt.bfloat16
    fp8 = mybir.dt.float8e4
    i32 = mybir.dt.int32
    u16 = mybir.dt.uint16
    Tanh = mybir.ActivationFunctionType.Tanh
    Exp = mybir.ActivationFunctionType.Exp
    DR = mybir.MatmulPerfMode.DoubleRow

    n_et = e_dim // 128            # e 128-tiles (contraction for e_proj)
    n_dt = d_dim // 128            # d 128-tiles (e_proj output tiles)
    n_gt = P_PAD // 128            # compact s-tiles per batch row (9)
    n_kk = n_et // 2               # DoubleRow K=256 steps
    # (chunk, tile-within-chunk) for each global compact tile
    tile_map = []
    for c, csz in enumerate(CHUNKS):
        for jj in range(csz // 128):
            tile_map.append((c, jj))

    nc = bacc.Bacc("TRN2", target_bir_lowering=False, debug=debug)

    hs_h = nc.dram_tensor("hidden_states", [bc, d_dim], f32, kind="ExternalInput")
    enc_h = nc.dram_tensor("encoder_outputs", [bc, s, e_dim], f32, kind="ExternalInput")
    gidx_h = nc.dram_tensor("gidx", [bc, 128, n_gt], i32, kind="ExternalInput")
    cbias_h = nc.dram_tensor("cbias", [bc, 128, n_gt], f32, kind="ExternalInput")
    ab_h = nc.dram_tensor("a_b", [d_dim], f32, kind="ExternalInput")
    vw_h = nc.dram_tensor("v_w", [d_dim], f32, kind="ExternalInput")
    wenc8_h = nc.dram_tensor("w_enc_fp8", [128, n_et, d_dim], fp8, kind="ExternalInput")
    wd8_h = nc.dram_tensor("w_dec_fp8", [128, n_dt, d_dim], fp8, kind="ExternalInput")
    id_h = nc.dram_tensor("ident", [128, 128], bf16, kind="ExternalInput")
    out_h = nc.dram_tensor("out", [bc, e_dim], f32, kind="ExternalOutput")

    enc_flat = enc_h[:, :, :].rearrange("b s e -> (b s) e")

    with tile.TileContext(nc) as tc, ExitStack() as ctx:
        consts = ctx.enter_context(tc.tile_pool(name="consts", bufs=1))
        enc_pool = ctx.enter_context(tc.tile_pool(name="enc", bufs=10))
        encT_pool = ctx.enter_context(tc.tile_pool(name="encT", bufs=4))
        th_pool = ctx.enter_context(tc.tile_pool(name="tanh", bufs=10))
        p_pool = ctx.enter_context(tc.tile_pool(name="p", bufs=2))
        small_pool = ctx.enter_context(tc.tile_pool(name="small", bufs=6))
        outsb_pool = ctx.enter_context(tc.tile_pool(name="outsb", bufs=2))
        pe_psum = ctx.enter_context(tc.tile_pool(name="pe_psum", bufs=2, space="PSUM"))
        tr_psum = ctx.enter_context(tc.tile_pool(name="tr_psum", bufs=2, space="PSUM"))
        sc_psum = ctx.enter_context(tc.tile_pool(name="sc_psum", bufs=1, space="PSUM"))
        w_psum = ctx.enter_context(tc.tile_pool(name="w_psum", bufs=1, space="PSUM"))

        # ---------------- prologue DMAs (transfers serialize; this order
        # is the pipeline-fill critical path) ----------------
        gidx_sb = consts.tile([128, bc, n_gt], i32)
        nc.sync.dma_start(out=gidx_sb, in_=gidx_h[:, :, :].rearrange("b p g -> p b g"))

        cbias_sb = consts.tile([128, bc, n_gt], f32)
        nc.sync.dma_start(
            out=cbias_sb, in_=cbias_h[:, :, :].rearrange("b p g -> p b g")
        )

        id_sb = consts.tile([128, 128], bf16)
        nc.sync.dma_start(out=id_sb, in_=id_h[:, :])

        hs_bf = consts.tile([bc, d_dim], bf16)
        nc.gpsimd.dma_start(out=hs_bf, in_=hs_h[:, :])  # cast f32->bf16

        enc_chunks = {}
        state = {}

        def emit_gather(b, c):
            """Gather unmasked encoder rows for chunk (b, c): one
            128-index SWDGE call per 128-token tile (the silicon-
            validated gather shape), f32->bf16 cast in the DMA."""
            enc_c = enc_pool.tile([128, 4, e_dim], bf16, tag="enc")
            g0 = sum(cs // 128 for cs in CHUNKS[:c])
            for jj in range(CHUNKS[c] // 128):
                nc.gpsimd.indirect_dma_start(
                    out=enc_c[:, jj, :],
                    out_offset=None,
                    in_=enc_flat,
                    in_offset=bass.IndirectOffsetOnAxis(
                        ap=gidx_sb[:, b, g0 + jj : g0 + jj + 1], axis=0
                    ),
                )
            enc_chunks[(b, c)] = enc_c

        emit_gather(0, 0)
        emit_gather(0, 1)
        emit_gather(0, 2)

        wenc8_sb = consts.tile([128, n_et, d_dim], fp8)
        nc.sync.dma_start(out=wenc8_sb, in_=wenc8_h[:, :, :])

        wd8_sb = consts.tile([128, n_dt, d_dim], fp8)
        nc.sync.dma_start(out=wd8_sb, in_=wd8_h[:, :, :])

        emit_gather(1, 0)
        emit_gather(1, 1)
        emit_gather(1, 2)
        v_sb = consts.tile([128, n_dt], bf16)
        nc.gpsimd.dma_start(out=v_sb, in_=vw_h[:].rearrange("(i p) -> p i", p=128))

        ab_sb = consts.tile([128, n_dt], f32)
        nc.sync.dma_start(out=ab_sb, in_=ab_h[:].rearrange("(i p) -> p i", p=128))

        ones_col = consts.tile([128, 1], f32)
        nc.vector.memset(ones_col, 1.0)
        ones_row = consts.tile([1, 128], f32)
        nc.vector.memset(ones_row, 1.0)
        ones4 = consts.tile([128, bc], f32)
        nc.vector.memset(ones4, 1.0)
        # a_b broadcast to (d-tile, b) layout: ab_rep[p, i, :] = a_b[128i+p]
        ab_rep = consts.tile([128, n_dt, bc], f32)
        for i in range(n_dt):
            nc.vector.tensor_scalar_mul(ab_rep[:, i, :], ones4, ab_sb[:, i : i + 1])

        # ---------------- h_proj (tiny; emitted via mid-hook inside the
        # first e_proj so the in-order PE queue isn't head-blocked while
        # w_dec_fp8 is still in flight) ----------------
        hb_sb = consts.tile([128, n_dt, bc], f32)

        def emit_hproj():
            # hiddenT (d on partitions) via K=bc transpose-by-matmul.
            # PSUM comes from tr_psum: pe_psum buffers hold un-evacuated
            # e_proj output whose tanh waits on hb -> using them here
            # would deadlock the PE queue. All accumulation stays in one
            # PSUM tile (groups strictly sequential per i-block) so the
            # critical chain is 2 engine hops: ScalarE hT8 cast + one
            # fused DVE scale-and-bias.
            psum_h = tr_psum.tile([128, n_dt * bc], f32, tag="tr")
            for k in range(n_dt):
                nc.tensor.matmul(
                    psum_h[:, bc * k : bc * (k + 1)],
                    lhsT=hs_bf[:, 128 * k : 128 * (k + 1)],
                    rhs=id_sb[0:bc, 0:bc],
                    start=True,
                    stop=True,
                )
            hT8 = consts.tile([128, n_dt, bc], fp8)
            nc.vector.tensor_copy(hT8, psum_h)

            # per-k PSUM groups + f32 SBUF accumulation (the silicon-
            # validated shape; a single in-PSUM accumulation would be
            # cheaper but its bank-wide has_written interplay is unproven
            # on hardware)
            hacc = consts.tile([128, n_dt * bc], f32)
            for k in range(n_dt):
                psum_hp = tr_psum.tile([128, n_dt * bc], f32, tag="tr")
                for i in range(n_dt):
                    nc.tensor.matmul(
                        psum_hp[:, bc * i : bc * (i + 1)],
                        lhsT=wd8_sb[:, k, 128 * i : 128 * (i + 1)],
                        rhs=hT8[:, k, :],
                        start=True,
                        stop=True,
                    )
                if k == 0:
                    nc.vector.tensor_copy(hacc, psum_hp)
                else:
                    nc.vector.tensor_add(hacc, hacc, psum_hp)
            # hb = hacc / W_SCALE + a_b (weights were pre-scaled *64)
            nc.vector.scalar_tensor_tensor(
                hb_sb.rearrange("p a b -> p (a b)"),
                hacc,
                1.0 / W_SCALE,
                ab_rep.rearrange("p a b -> p (a b)"),
                op0=mybir.AluOpType.mult,
                op1=mybir.AluOpType.add,
            )

        # ---------------- per-chunk stages ----------------

        def emit_transpose_j(b, c, j):
            """One 128-token tile of encT for chunk (b, c): 8 PE
            transposes (all e-tiles of tile j) into a PSUM bank + one
            cast-evacuation (ScalarE where it would otherwise idle, DVE
            steady; GpSimd has no PSUM port). Per-tile units mean a unit
            only waits on its own gather op."""
            if (b, c) not in state:
                state[(b, c)] = encT_pool.tile(
                    [128, n_et, 512], fp8, tag="encT", name="encT8"
                )
            encT8 = state[(b, c)]
            chunk = enc_chunks[(b, c)]
            tp = tr_psum.tile([128, n_et, 128], bf16, tag="tr", name="tp")
            for et in range(n_et):
                nc.tensor.transpose(
                    tp[:, et, :],
                    chunk[:, j, 128 * et : 128 * (et + 1)],
                    id_sb,
                )
            dst = encT8[:, :, 128 * j : 128 * (j + 1)]
            nc.vector.tensor_copy(dst, tp)

        def emit_transposes(b, c):
            for j in range(CHUNKS[c] // 128):
                emit_transpose_j(b, c, j)

        def emit_eproj_pair(b, mid_hook=None):
            # chunks 0+1 together: tanh runs on [128, 1024] tiles (one
            # ScalarE init per two chunks); the two 512-wide matmul
            # groups land in the two banks of a 2-bank PSUM tile.
            eTa = state.pop((b, 0))
            eTb = state.pop((b, 1))
            state[("sc", b)] = sc_psum.tile([128, n_gt], f32, tag="sc", name="psc")
            ths = []
            for i in range(n_dt):
                pe = pe_psum.tile([128, 2, 512], f32, tag="pe")
                for half, eT in ((0, eTa), (1, eTb)):
                    for kk in range(n_kk):
                        nc.tensor.matmul(
                            pe[:, half, :],
                            lhsT=wenc8_sb[
                                :, 2 * kk : 2 * kk + 2, 128 * i : 128 * (i + 1)
                            ],
                            rhs=eT[:, 2 * kk : 2 * kk + 2, :],
                            start=(kk == 0),
                            stop=(kk == n_kk - 1),
                            perf_mode=DR,
                        )
                if mid_hook is not None and i == 1:
                    mid_hook()
                    mid_hook = None
                th = th_pool.tile([128, 2, 512], bf16, tag="tanh")
                nc.scalar.activation(
                    th.rearrange("p a b -> p (a b)"),
                    pe.rearrange("p a b -> p (a b)"),
                    Tanh,
                    bias=hb_sb[:, i, b : b + 1],
                    scale=1.0 / W_SCALE,
                )
                ths.append(th)
            state[("th", b)] = ths

        def emit_eproj_tail(b):
            # chunk 2: single 128-token tile
            eT = state.pop((b, 2))
            ths = []
            for i in range(n_dt):
                pe = pe_psum.tile([128, 2, 512], f32, tag="pe")
                for kk in range(n_kk):
                    nc.tensor.matmul(
                        pe[:, 0, 0:128],
                        lhsT=wenc8_sb[:, 2 * kk : 2 * kk + 2, 128 * i : 128 * (i + 1)],
                        rhs=eT[:, 2 * kk : 2 * kk + 2, 0:128],
                        start=(kk == 0),
                        stop=(kk == n_kk - 1),
                        perf_mode=DR,
                    )
                th = th_pool.tile([128, 128], bf16, tag="ttail")
                nc.scalar.activation(
                    th, pe[:, 0, 0:128], Tanh, bias=hb_sb[:, i, b : b + 1],
                    scale=1.0 / W_SCALE,
                )
                ths.append(th)
            state[("tht", b)] = ths

        scores_done = {}

        def emit_scores_col(b, col):
            # Column-outer, i-inner: accumulation groups in the scoresT
            # bank must be strictly sequential (start=True clears
            # has_written for the WHOLE bank).
            ths = state[("th", b)]
            psum_sc = state[("sc", b)]
            half, jj = divmod(col, 4)
            for i in range(n_dt):
                nc.tensor.matmul(
                    psum_sc[:, col : col + 1],
                    lhsT=ths[i][:, half, 128 * jj : 128 * (jj + 1)],
                    rhs=v_sb[:, i : i + 1],
                    start=(i == 0),
                    stop=(i == n_dt - 1),
                )
            scores_done[b] = scores_done.get(b, 0) + 1

        def emit_scores_pair(b):
            for col in range(scores_done.get(b, 0), 8):
                emit_scores_col(b, col)
            state.pop(("th", b))

        def emit_scores_tail(b):
            ths = state.pop(("tht", b))
            psum_sc = state[("sc", b)]
            for i in range(n_dt):
                nc.tensor.matmul(
                    psum_sc[:, 8:9],
                    lhsT=ths[i],
                    rhs=v_sb[:, i : i + 1],
                    start=(i == 0),
                    stop=(i == n_dt - 1),
                )

        def emit_softmax_a(b):
            """Compact-mask bias + exp with fused row-sums (DVE+ScalarE)."""
            psum_sc = state.pop(("sc", b))
            nc.vector.tensor_add(psum_sc, psum_sc, cbias_sb[:, b, :])
            p_bf = p_pool.tile([128, n_gt], bf16, tag="p")
            rowsum = small_pool.tile([128, 1], f32, tag="rowsum")
            nc.scalar.activation(
                p_bf, psum_sc, Exp, bias=0.0, scale=1.0, accum_out=rowsum
            )
            state[("p", b)] = p_bf
            state[("rowsum", b)] = rowsum

        def emit_ssum_recip(b):
            rowsum = state.pop(("rowsum", b))
            ssum = w_psum.tile([1, 1], f32, tag="w")
            nc.tensor.matmul(ssum, lhsT=rowsum, rhs=ones_col, start=True, stop=True)
            rsum = small_pool.tile([1, 1], f32, tag="rsum")
            nc.vector.reciprocal(rsum, ssum)
            state[("rsum", b)] = rsum

        def emit_weighted(b):
            p_bf = state.pop(("p", b))
            rsum = state.pop(("rsum", b))
            rbc_ps = w_psum.tile([128, 1], f32, tag="w")
            nc.tensor.matmul(rbc_ps, lhsT=ones_row, rhs=rsum, start=True, stop=True)
            rbc = small_pool.tile([128, 1], f32, tag="rbc")
            nc.vector.tensor_copy(rbc, rbc_ps)
            w_ps = w_psum.tile([128, n_dt], f32, tag="w")
            for i in range(n_et):
                for g, (c, jj) in enumerate(tile_map):
                    nc.tensor.matmul(
                        w_ps[:, i : i + 1],
                        lhsT=enc_chunks[(b, c)][:, jj, 128 * i : 128 * (i + 1)],
                        rhs=p_bf[:, g : g + 1],
                        start=(g == 0),
                        stop=(g == n_gt - 1),
                    )
            for c in range(len(CHUNKS)):
                del enc_chunks[(b, c)]
            out_sb = outsb_pool.tile([128, n_et], f32, tag="outsb")
            nc.vector.tensor_scalar_mul(out_sb, w_ps, rbc[:, 0:1])
            nc.sync.dma_start(
                out=out_h[b, :].rearrange("(i p) -> p i", p=128), in_=out_sb
            )

        # ---------------- schedule ----------------
        # Two sub-stages per batch row: A(b) = chunks 0+1 e_proj, B(b) =
        # tail e_proj. Transposes run one sub-stage ahead of their
        # e_proj, scores one sub-stage behind, so the in-order PE queue
        # never blocks on ScalarE/DVE results.
        emit_transposes(0, 0)
        emit_transposes(0, 1)
        for b in range(bc):
            # --- sub-stage A(b) ---
            if b + 2 < bc:
                emit_gather(b + 2, 0)
                emit_gather(b + 2, 1)
            emit_eproj_pair(b, mid_hook=emit_hproj if b == 0 else None)
            emit_transposes(b, 2)
            if b > 0:
                emit_scores_tail(b - 1)
                emit_softmax_a(b - 1)
            # --- sub-stage B(b) ---
            if b + 2 < bc:
                emit_gather(b + 2, 2)
            emit_eproj_tail(b)
            if b + 1 < bc:
                emit_transposes(b + 1, 0)
                emit_transposes(b + 1, 1)
            emit_scores_pair(b)
            if b > 0:
                emit_ssum_recip(b - 1)
                emit_weighted(b - 1)
        emit_scores_tail(bc - 1)
        emit_softmax_a(bc - 1)
        emit_ssum_recip(bc - 1)
        emit_weighted(bc - 1)

    nc.compile()
    return nc


_CACHE = {}


def _prep_weights(a_w):
    """Host-side weight repack: w_enc and w_dec scaled by 64 and
    quantized to fp8e4m3 in (p, k, d) layout matching the stationary-
    operand slices (DoubleRow pairs for w_enc)."""
    import ml_dtypes

    def pack(w):
        w = (np.asarray(w, dtype=np.float32) * W_SCALE).reshape(-1, 128, DEC)
        return np.ascontiguousarray(w.transpose(1, 0, 2)).astype(
            ml_dtypes.float8_e4m3
        )

    return pack(a_w[DEC:]), pack(a_w[:DEC])


def _prep_indices(masks):
    """Per-row unmasked token indices (padded to P_PAD with row 0 of the
    same batch row - its lanes are killed by cbias) and the compact-mask
    bias, both in column-major (p, g) tile layout."""
    bc = masks.shape[0]
    gidx = np.zeros((bc, P_PAD), dtype=np.int32)
    cbias = np.full((bc, P_PAD), -1e10, dtype=np.float32)
    for b in range(bc):
        idx = np.nonzero(masks[b])[0].astype(np.int32)
        cnt = len(idx)
        assert cnt <= P_PAD, f"unmasked count {cnt} exceeds P_PAD={P_PAD}"
        gidx[b, :cnt] = b * S + idx
        gidx[b, cnt:] = b * S
        cbias[b, :cnt] = 0.0
    # (b, tile*128 + p) -> (b, p, tile)
    gidx = np.ascontiguousarray(gidx.reshape(bc, P_PAD // 128, 128).transpose(0, 2, 1))
    cbias = np.ascontiguousarray(
        cbias.reshape(bc, P_PAD // 128, 128).transpose(0, 2, 1)
    )
    return gidx, cbias


def kernel(hidden_states, encoder_outputs, encoder_masks, a_w, a_b, v_w):
    import ml_dtypes
    from concourse.bass_utils import run_bass_kernel_spmd

    if "nc" not in _CACHE:
        _CACHE["nc"] = build_bass_kernel()
    nc = _CACHE["nc"]

    hidden_states = np.asarray(hidden_states, dtype=np.float32)
    encoder_outputs = np.asarray(encoder_outputs, dtype=np.float32)
    encoder_masks = np.asarray(encoder_masks, dtype=np.int32)
    a_w = np.ascontiguousarray(np.asarray(a_w, dtype=np.float32))
    a_b = np.ascontiguousarray(np.asarray(a_b, dtype=np.float32))
    v_w = np.ascontiguousarray(np.asarray(v_w, dtype=np.float32))
    ident = np.eye(128, dtype=ml_dtypes.bfloat16)
    wenc8, wd8 = _prep_weights(a_w)

    in_maps = []
    for c in range(N_CORES):
        sl = slice(c * BC, (c + 1) * BC)
        gidx, cbias = _prep_indices(encoder_masks[sl])
        in_maps.append(
            {
                "hidden_states": np.ascontiguousarray(hidden_states[sl]),
                "encoder_outputs": np.ascontiguousarray(encoder_outputs[sl]),
                "gidx": gidx,
                "cbias": cbias,
                "a_b": a_b,
                "v_w": v_w,
                "w_enc_fp8": wenc8,
                "w_dec_fp8": wd8,
                "ident": ident,
            }
        )

    global _LAST_IN_MAPS
    _LAST_IN_MAPS = in_maps
    res = run_bass_kernel_spmd(nc, in_maps, core_ids=list(range(N_CORES)))
    out = np.concatenate([r["out"] for r in res.results], axis=0)
    return out.astype(np.float32)


_LAST_IN_MAPS = None


# revision 30
# speedup vs baseline: 3.7377x; 1.0275x over previous
"""Bahdanau-style attention kernel for Trainium2 (8 NeuronCores, SPMD).

Math (per batch row b):
    h_proj = hidden @ a_w[:DEC]                       (DEC,)
    e_proj[s, :] = enc[s, :] @ a_w[DEC:]              (S, DEC)
    energy = tanh(e_proj + h_proj + a_b)              (S, DEC)
    scores = energy @ v_w                             (S,)
    scores = where(mask == 0, -1e10, scores)
    attn = softmax(scores)                            (S,)
    out = attn @ enc                                  (ENC,)

Sharding: data-parallel over batch (32 rows -> 4 rows on each of 8 cores);
weights replicated.

Masked tokens get attn == 0 exactly, so only the unmasked rows (~half;
Binomial(2048, .5), padded to P_PAD=1152 = +5.7 sigma) contribute to any
output. The host computes each row's unmasked-index list (cheap metadata,
<0.01% of the FLOPs - the kernel-side equivalent was measured
descriptor-bound on Q7) and the device gathers just those encoder rows
with indirect SWDGE DMAs (fp32->bf16 cast in flight, one 128-index call
per tile - the silicon-validated gather shape). Pad lanes are killed by
a host-built -1e10 compact-mask bias, so the math is exactly the
reference's masked softmax.

Per-core pipeline per batch row (9 compact 128-token tiles as chunks of
512/512/128):
  - encT built by PE transpose-mode matmuls (128x128 tiles) into PSUM
    (bf16), evacuated to SBUF with a fused bf16->fp8e4m3 cast on DVE
    (ScalarE fp8 casts measured noisier on silicon; GpSimd has no PSUM
    port).
  - e_proj transposed (d on partitions) with fp8 DoubleRow matmuls
    (K=256 per instruction): lhsT = host-prequantized w_enc * 64 fp8,
    rhs = encT fp8 pairs. The 1/64 rescale and (h_proj + a_b) ride the
    tanh activation's scale/bias; tanh runs on [128, 1024] tiles (chunk
    pairs) to halve the per-op ScalarE init cost.
  - scores = v . tanh as columns: N=1 matmuls, th 128x128 slices
    stationary, v column moving -> scoresT in a [128, 9] PSUM tile
    (accumulation groups strictly sequential per column - start=True
    clears has_written bank-wide).
  - softmax unnormalized: compact-mask bias added to scoresT PSUM, Exp
    on ScalarE with accum_out row-sums, denominator closed by one
    cross-partition N=1 matmul; the 1/sum rescale lands once on the
    final weighted sum.
  - weighted sum as N=1 matmuls: lhsT = natural-layout gathered rows
    (bf16, unquantized - fp8 enc here would put ~4% noise on the
    output), rhs = p column.
"""

import numpy as np
from contextlib import ExitStack

B, S, ENC, DEC = 32, 2048, 1024, 1024
N_CORES = 8
BC = B // N_CORES   # batch rows per core
W_SCALE = 64.0      # fp8 weight pre-scale (avoids e4m3 subnormal range)
# padded compact-token count: Binomial(2048, 0.5) is 1024 +- 22.6, so 1152
# is a +5.7 sigma bound (seed-0 data maxes at 1062)
P_PAD = 1152
CHUNKS = (512, 512, 128)   # compact tokens per chunk (= 9 tiles of 128)


def build_bass_kernel(bc=BC, s=S, e_dim=ENC, d_dim=DEC, debug=False):
    import concourse.bass as bass
    import concourse.tile as tile
    from concourse import bacc, mybir

    f32 = mybir.dt.float32
    bf16 = mybir.dt.bfloat16
    fp8 = mybir.dt.float8e4
    i32 = mybir.dt.int32
    u16 = mybir.dt.uint16
    Tanh = mybir.ActivationFunctionType.Tanh
    Exp = mybir.ActivationFunctionType.Exp
    DR = mybir.MatmulPerfMode.DoubleRow

    n_et = e_dim // 128            # e 128-tiles (contraction for e_proj)
    n_dt = d_dim // 128            # d 128-tiles (e_proj output tiles)
    n_gt = P_PAD // 128            # compact s-tiles per batch row (9)
    n_kk = n_et // 2               # DoubleRow K=256 steps
    # (chunk, tile-within-chunk) for each global compact tile
    tile_map = []
    for c, csz in enumerate(CHUNKS):
        for jj in range(csz // 128):
            tile_map.append((c, jj))

    nc = bacc.Bacc("TRN2", target_bir_lowering=False, debug=debug)

    hs_h = nc.dram_tensor("hidden_states", [bc, d_dim], f32, kind="ExternalInput")
    enc_h = nc.dram_tensor("encoder_outputs", [bc, s, e_dim], f32, kind="ExternalInput")
    gidx_h = nc.dram_tensor("gidx", [bc, 128, n_gt], i32, kind="ExternalInput")
    cbias_h = nc.dram_tensor("cbias", [bc, 128, n_gt], f32, kind="ExternalInput")
    ab_h = nc.dram_tensor("a_b", [d_dim], f32, kind="ExternalInput")
    vw_h = nc.dram_tensor("v_w", [d_dim], f32, kind="ExternalInput")
    wenc8_h = nc.dram_tensor("w_enc_fp8", [128, n_et, d_dim], fp8, kind="ExternalInput")
    wd8_h = nc.dram_tensor("w_dec_fp8", [128, n_dt, d_dim], fp8, kind="ExternalInput")
    id_h = nc.dram_tensor("ident", [128, 128], bf16, kind="ExternalInput")
    out_h = nc.dram_tensor("out", [bc, e_dim], f32, kind="ExternalOutput")

    enc_flat = enc_h[:, :, :].rearrange("b s e -> (b s) e")

    with tile.TileContext(nc) as tc, ExitStack() as ctx:
        consts = ctx.enter_context(tc.tile_pool(name="consts", bufs=1))
        enc_pool = ctx.enter_context(tc.tile_pool(name="enc", bufs=10))
        encT_pool = ctx.enter_context(tc.tile_pool(name="encT", bufs=4))
        th_pool = ctx.enter_context(tc.tile_pool(name="tanh", bufs=10))
        p_pool = ctx.enter_context(tc.tile_pool(name="p", bufs=2))
        small_pool = ctx.enter_context(tc.tile_pool(name="small", bufs=6))
        outsb_pool = ctx.enter_context(tc.tile_pool(name="outsb", bufs=2))
        pe_psum = ctx.enter_context(tc.tile_pool(name="pe_psum", bufs=2, space="PSUM"))
        tr_psum = ctx.enter_context(tc.tile_pool(name="tr_psum", bufs=2, space="PSUM"))
        sc_psum = ctx.enter_context(tc.tile_pool(name="sc_psum", bufs=1, space="PSUM"))
        w_psum = ctx.enter_context(tc.tile_pool(name="w_psum", bufs=1, space="PSUM"))

        # ---------------- prologue DMAs (transfers serialize; this order
        # is the pipeline-fill critical path) ----------------
        gidx_sb = consts.tile([128, bc, n_gt], i32)
        nc.sync.dma_start(out=gidx_sb, in_=gidx_h[:, :, :].rearrange("b p g -> p b g"))

        cbias_sb = consts.tile([128, bc, n_gt], f32)
        nc.sync.dma_start(
            out=cbias_sb, in_=cbias_h[:, :, :].rearrange("b p g -> p b g")
        )

        id_sb = consts.tile([128, 128], bf16)
        nc.sync.dma_start(out=id_sb, in_=id_h[:, :])

        hs_bf = consts.tile([bc, d_dim], bf16)
        nc.gpsimd.dma_start(out=hs_bf, in_=hs_h[:, :])  # cast f32->bf16

        enc_chunks = {}
        state = {}

        def emit_gather(b, c):
            """Gather unmasked encoder rows for chunk (b, c): one
            128-index SWDGE call per 128-token tile (the silicon-
            validated gather shape), f32->bf16 cast in the DMA."""
            enc_c = enc_pool.tile([128, 4, e_dim], bf16, tag="enc")
            g0 = sum(cs // 128 for cs in CHUNKS[:c])
            for jj in range(CHUNKS[c] // 128):
                nc.gpsimd.indirect_dma_start(
                    out=enc_c[:, jj, :],
                    out_offset=None,
                    in_=enc_flat,
                    in_offset=bass.IndirectOffsetOnAxis(
                        ap=gidx_sb[:, b, g0 + jj : g0 + jj + 1], axis=0
                    ),
                )
            enc_chunks[(b, c)] = enc_c

        emit_gather(0, 0)
        emit_gather(0, 1)
        emit_gather(0, 2)

        wenc8_sb = consts.tile([128, n_et, d_dim], fp8)
        nc.sync.dma_start(out=wenc8_sb, in_=wenc8_h[:, :, :])

        wd8_sb = consts.tile([128, n_dt, d_dim], fp8)
        nc.sync.dma_start(out=wd8_sb, in_=wd8_h[:, :, :])

        emit_gather(1, 0)
        emit_gather(1, 1)
        emit_gather(1, 2)
        v_sb = consts.tile([128, n_dt], bf16)
        nc.gpsimd.dma_start(out=v_sb, in_=vw_h[:].rearrange("(i p) -> p i", p=128))

        ab_sb = consts.tile([128, n_dt], f32)
        nc.sync.dma_start(out=ab_sb, in_=ab_h[:].rearrange("(i p) -> p i", p=128))

        ones_col = consts.tile([128, 1], f32)
        nc.vector.memset(ones_col, 1.0)
        ones_row = consts.tile([1, 128], f32)
        nc.vector.memset(ones_row, 1.0)
        ones4 = consts.tile([128, bc], f32)
        nc.vector.memset(ones4, 1.0)
        # a_b broadcast to (d-tile, b) layout: ab_rep[p, i, :] = a_b[128i+p]
        ab_rep = consts.tile([128, n_dt, bc], f32)
        for i in range(n_dt):
            nc.vector.tensor_scalar_mul(ab_rep[:, i, :], ones4, ab_sb[:, i : i + 1])

        # ---------------- h_proj (tiny; emitted via mid-hook inside the
        # first e_proj so the in-order PE queue isn't head-blocked while
        # w_dec_fp8 is still in flight) ----------------
        hb_sb = consts.tile([128, n_dt, bc], f32)

        def emit_hproj():
            # hiddenT (d on partitions) via K=bc transpose-by-matmul.
            # PSUM comes from tr_psum: pe_psum buffers hold un-evacuated
            # e_proj output whose tanh waits on hb -> using them here
            # would deadlock the PE queue. All accumulation stays in one
            # PSUM tile (groups strictly sequential per i-block) so the
            # critical chain is 2 engine hops: ScalarE hT8 cast + one
            # fused DVE scale-and-bias.
            psum_h = tr_psum.tile([128, n_dt * bc], f32, tag="tr")
            for k in range(n_dt):
                nc.tensor.matmul(
                    psum_h[:, bc * k : bc * (k + 1)],
                    lhsT=hs_bf[:, 128 * k : 128 * (k + 1)],
                    rhs=id_sb[0:bc, 0:bc],
                    start=True,
                    stop=True,
                )
            hT8 = consts.tile([128, n_dt, bc], fp8)
            nc.vector.tensor_copy(hT8, psum_h)

            # single-PSUM accumulation: per-i-block groups run strictly
            # sequentially in one bank (start=True clears has_written
            # bank-wide but leaves data; closed blocks are never
            # re-accumulated)
            psum_hp = tr_psum.tile([128, n_dt * bc], f32, tag="tr")
            for i in range(n_dt):
                for k in range(n_dt):
                    nc.tensor.matmul(
                        psum_hp[:, bc * i : bc * (i + 1)],
                        lhsT=wd8_sb[:, k, 128 * i : 128 * (i + 1)],
                        rhs=hT8[:, k, :],
                        start=(k == 0),
                        stop=(k == n_dt - 1),
                    )
            # hb = psum / W_SCALE + a_b (weights were pre-scaled *64)
            nc.vector.scalar_tensor_tensor(
                hb_sb.rearrange("p a b -> p (a b)"),
                psum_hp,
                1.0 / W_SCALE,
                ab_rep.rearrange("p a b -> p (a b)"),
                op0=mybir.AluOpType.mult,
                op1=mybir.AluOpType.add,
            )

        # ---------------- per-chunk stages ----------------

        def emit_transpose_j(b, c, j):
            """One 128-token tile of encT for chunk (b, c): 8 PE
            transposes (all e-tiles of tile j) into a PSUM bank + one
            cast-evacuation (ScalarE where it would otherwise idle, DVE
            steady; GpSimd has no PSUM port). Per-tile units mean a unit
            only waits on its own gather op."""
            if (b, c) not in state:
                state[(b, c)] = encT_pool.tile(
                    [128, n_et, 512], fp8, tag="encT", name="encT8"
                )
            encT8 = state[(b, c)]
            chunk = enc_chunks[(b, c)]
            tp = tr_psum.tile([128, n_et, 128], bf16, tag="tr", name="tp")
            for et in range(n_et):
                nc.tensor.transpose(
                    tp[:, et, :],
                    chunk[:, j, 128 * et : 128 * (et + 1)],
                    id_sb,
                )
            dst = encT8[:, :, 128 * j : 128 * (j + 1)]
            nc.vector.tensor_copy(dst, tp)

        def emit_transposes(b, c):
            for j in range(CHUNKS[c] // 128):
                emit_transpose_j(b, c, j)

        def emit_eproj_pair(b, mid_hook=None):
            # chunks 0+1 together: tanh runs on [128, 1024] tiles (one
            # ScalarE init per two chunks); the two 512-wide matmul
            # groups land in the two banks of a 2-bank PSUM tile.
            eTa = state.pop((b, 0))
            eTb = state.pop((b, 1))
            state[("sc", b)] = sc_psum.tile([128, n_gt], f32, tag="sc", name="psc")
            ths = []
            for i in range(n_dt):
                pe = pe_psum.tile([128, 2, 512], f32, tag="pe")
                for half, eT in ((0, eTa), (1, eTb)):
                    for kk in range(n_kk):
                        nc.tensor.matmul(
                            pe[:, half, :],
                            lhsT=wenc8_sb[
                                :, 2 * kk : 2 * kk + 2, 128 * i : 128 * (i + 1)
                            ],
                            rhs=eT[:, 2 * kk : 2 * kk + 2, :],
                            start=(kk == 0),
                            stop=(kk == n_kk - 1),
                            perf_mode=DR,
                        )
                if mid_hook is not None and i == 1:
                    mid_hook()
                    mid_hook = None
                th = th_pool.tile([128, 2, 512], bf16, tag="tanh")
                nc.scalar.activation(
                    th.rearrange("p a b -> p (a b)"),
                    pe.rearrange("p a b -> p (a b)"),
                    Tanh,
                    bias=hb_sb[:, i, b : b + 1],
                    scale=1.0 / W_SCALE,
                )
                ths.append(th)
            state[("th", b)] = ths

        def emit_eproj_tail(b):
            # chunk 2: single 128-token tile
            eT = state.pop((b, 2))
            ths = []
            for i in range(n_dt):
                pe = pe_psum.tile([128, 2, 512], f32, tag="pe")
                for kk in range(n_kk):
                    nc.tensor.matmul(
                        pe[:, 0, 0:128],
                        lhsT=wenc8_sb[:, 2 * kk : 2 * kk + 2, 128 * i : 128 * (i + 1)],
                        rhs=eT[:, 2 * kk : 2 * kk + 2, 0:128],
                        start=(kk == 0),
                        stop=(kk == n_kk - 1),
                        perf_mode=DR,
                    )
                th = th_pool.tile([128, 128], bf16, tag="ttail")
                nc.scalar.activation(
                    th, pe[:, 0, 0:128], Tanh, bias=hb_sb[:, i, b : b + 1],
                    scale=1.0 / W_SCALE,
                )
                ths.append(th)
            state[("tht", b)] = ths

        scores_done = {}

        def emit_scores_col(b, col):
            # Column-outer, i-inner: accumulation groups in the scoresT
            # bank must be strictly sequential (start=True clears
            # has_written for the WHOLE bank).
            ths = state[("th", b)]
            psum_sc = state[("sc", b)]
            half, jj = divmod(col, 4)
            for i in range(n_dt):
                nc.tensor.matmul(
                    psum_sc[:, col : col + 1],
                    lhsT=ths[i][:, half, 128 * jj : 128 * (jj + 1)],
                    rhs=v_sb[:, i : i + 1],
                    start=(i == 0),
                    stop=(i == n_dt - 1),
                )
            scores_done[b] = scores_done.get(b, 0) + 1

        def emit_scores_pair(b):
            for col in range(scores_done.get(b, 0), 8):
                emit_scores_col(b, col)
            state.pop(("th", b))

        def emit_scores_tail(b):
            ths = state.pop(("tht", b))
            psum_sc = state[("sc", b)]
            for i in range(n_dt):
                nc.tensor.matmul(
                    psum_sc[:, 8:9],
                    lhsT=ths[i],
                    rhs=v_sb[:, i : i + 1],
                    start=(i == 0),
                    stop=(i == n_dt - 1),
                )

        def emit_softmax_a(b):
            """Compact-mask bias + exp with fused row-sums (DVE+ScalarE)."""
            psum_sc = state.pop(("sc", b))
            nc.vector.tensor_add(psum_sc, psum_sc, cbias_sb[:, b, :])
            p_bf = p_pool.tile([128, n_gt], bf16, tag="p")
            rowsum = small_pool.tile([128, 1], f32, tag="rowsum")
            nc.scalar.activation(
                p_bf, psum_sc, Exp, bias=0.0, scale=1.0, accum_out=rowsum
            )
            state[("p", b)] = p_bf
            state[("rowsum", b)] = rowsum

        def emit_ssum_recip(b):
            rowsum = state.pop(("rowsum", b))
            ssum = w_psum.tile([1, 1], f32, tag="w")
            nc.tensor.matmul(ssum, lhsT=rowsum, rhs=ones_col, start=True, stop=True)
            rsum = small_pool.tile([1, 1], f32, tag="rsum")
            nc.vector.reciprocal(rsum, ssum)
            state[("rsum", b)] = rsum

        def emit_weighted(b):
            p_bf = state.pop(("p", b))
            rsum = state.pop(("rsum", b))
            rbc_ps = w_psum.tile([128, 1], f32, tag="w")
            nc.tensor.matmul(rbc_ps, lhsT=ones_row, rhs=rsum, start=True, stop=True)
            rbc = small_pool.tile([128, 1], f32, tag="rbc")
            nc.vector.tensor_copy(rbc, rbc_ps)
            w_ps = w_psum.tile([128, n_dt], f32, tag="w")
            for i in range(n_et):
                for g, (c, jj) in enumerate(tile_map):
                    nc.tensor.matmul(
                        w_ps[:, i : i + 1],
                        lhsT=enc_chunks[(b, c)][:, jj, 128 * i : 128 * (i + 1)],
                        rhs=p_bf[:, g : g + 1],
                        start=(g == 0),
                        stop=(g == n_gt - 1),
                    )
            for c in range(len(CHUNKS)):
                del enc_chunks[(b, c)]
            out_sb = outsb_pool.tile([128, n_et], f32, tag="outsb")
            nc.vector.tensor_scalar_mul(out_sb, w_ps, rbc[:, 0:1])
            nc.sync.dma_start(
                out=out_h[b, :].rearrange("(i p) -> p i", p=128), in_=out_sb
            )

        # ---------------- schedule ----------------
        # Two sub-stages per batch row: A(b) = chunks 0+1 e_proj, B(b) =
        # tail e_proj. Transposes run one sub-stage ahead of their
        # e_proj, scores one sub-stage behind, so the in-order PE queue
        # never blocks on ScalarE/DVE results.
        emit_transposes(0, 0)
        emit_transposes(0, 1)
        for b in range(bc):
            # --- sub-stage A(b) ---
            if b + 2 < bc:
                emit_gather(b + 2, 0)
                emit_gather(b + 2, 1)
            emit_eproj_pair(b, mid_hook=emit_hproj if b == 0 else None)
            emit_transposes(b, 2)
            if b > 0:
                emit_scores_tail(b - 1)
                emit_softmax_a(b - 1)
            # --- sub-stage B(b) ---
            if b + 2 < bc:
                emit_gather(b + 2, 2)
            emit_eproj_tail(b)
            if b + 1 < bc:
                emit_transposes(b + 1, 0)
                emit_transposes(b + 1, 1)
            emit_scores_pair(b)
            if b > 0:
                emit_ssum_recip(b - 1)
                emit_weighted(b - 1)
        emit_scores_tail(bc - 1)
        emit_softmax_a(bc - 1)
        emit_ssum_recip(bc - 1)
        emit_weighted(bc - 1)

    nc.compile()
    return nc


_CACHE = {}


def _prep_weights(a_w):
    """Host-side weight repack: w_enc and w_dec scaled by 64 and
    quantized to fp8e4m3 in (p, k, d) layout matching the stationary-
    operand slices (DoubleRow pairs for w_enc)."""
    import ml_dtypes

    def pack(w):
        w = (np.asarray(w, dtype=np.float32) * W_SCALE).reshape(-1, 128, DEC)
        return np.ascontiguousarray(w.transpose(1, 0, 2)).astype(
            ml_dtypes.float8_e4m3
        )

    return pack(a_w[DEC:]), pack(a_w[:DEC])


def _prep_indices(masks):
    """Per-row unmasked token indices (padded to P_PAD with row 0 of the
    same batch row - its lanes are killed by cbias) and the compact-mask
    bias, both in column-major (p, g) tile layout."""
    bc = masks.shape[0]
    gidx = np.zeros((bc, P_PAD), dtype=np.int32)
    cbias = np.full((bc, P_PAD), -1e10, dtype=np.float32)
    for b in range(bc):
        idx = np.nonzero(masks[b])[0].astype(np.int32)
        cnt = len(idx)
        assert cnt <= P_PAD, f"unmasked count {cnt} exceeds P_PAD={P_PAD}"
        gidx[b, :cnt] = b * S + idx
        gidx[b, cnt:] = b * S
        cbias[b, :cnt] = 0.0
    # (b, tile*128 + p) -> (b, p, tile)
    gidx = np.ascontiguousarray(gidx.reshape(bc, P_PAD // 128, 128).transpose(0, 2, 1))
    cbias = np.ascontiguousarray(
        cbias.reshape(bc, P_PAD // 128, 128).transpose(0, 2, 1)
    )
    return gidx, cbias


def kernel(hidden_states, encoder_outputs, encoder_masks, a_w, a_b, v_w):
    import ml_dtypes
    from concourse.bass_utils import run_bass_kernel_spmd

    if "nc" not in _CACHE:
        _CACHE["nc"] = build_bass_kernel()
    nc = _CACHE["nc"]

    hidden_states = np.asarray(hidden_states, dtype=np.float32)
    encoder_outputs = np.asarray(encoder_outputs, dtype=np.float32)
    encoder_masks = np.asarray(encoder_masks, dtype=np.int32)
    a_w = np.ascontiguousarray(np.asarray(a_w, dtype=np.float32))
    a_b = np.ascontiguousarray(np.asarray(a_b, dtype=np.float32))
    v_w = np.ascontiguousarray(np.asarray(v_w, dtype=np.float32))
    ident = np.eye(128, dtype=ml_dtypes.bfloat16)
    wenc8, wd8 = _prep_weights(a_w)

    in_maps = []
    for c in range(N_CORES):
        sl = slice(c * BC, (c + 1) * BC)
        gidx, cbias = _prep_indices(encoder_masks[sl])
        in_maps.append(
            {
                "hidden_states": np.ascontiguousarray(hidden_states[sl]),
                "encoder_outputs": np.ascontiguousarray(encoder_outputs[sl]),
                "gidx": gidx,
                "cbias": cbias,
                "a_b": a_b,
                "v_w": v_w,
                "w_enc_fp8": wenc8,
                "w_dec_fp8": wd8,
                "ident": ident,
            }
        )

    global _LAST_IN_MAPS
    _LAST_IN_MAPS = in_maps
    res = run_bass_kernel_spmd(nc, in_maps, core_ids=list(range(N_CORES)))
    out = np.concatenate([r["out"] for r in res.results], axis=0)
    return out.astype(np.float32)


_LAST_IN_MAPS = None


# revision 34
# speedup vs baseline: 3.9434x; 1.0550x over previous
"""Bahdanau-style attention kernel for Trainium2 (8 NeuronCores, SPMD).

Math (per batch row b):
    h_proj = hidden @ a_w[:DEC]                       (DEC,)
    e_proj[s, :] = enc[s, :] @ a_w[DEC:]              (S, DEC)
    energy = tanh(e_proj + h_proj + a_b)              (S, DEC)
    scores = energy @ v_w                             (S,)
    scores = where(mask == 0, -1e10, scores)
    attn = softmax(scores)                            (S,)
    out = attn @ enc                                  (ENC,)

Sharding: data-parallel over batch (32 rows -> 4 rows on each of 8 cores);
weights replicated.

Masked tokens get attn == 0 exactly, so only the unmasked rows (~half;
Binomial(2048, .5), padded to P_PAD=1152 = +5.7 sigma) contribute to any
output. The host computes each row's unmasked-index list (cheap metadata,
<0.01% of the FLOPs - the kernel-side equivalent was measured
descriptor-bound on Q7) and the device gathers just those encoder rows
with indirect SWDGE DMAs (fp32->bf16 cast in flight, one 128-index call
per tile - the silicon-validated gather shape). Pad lanes are killed by
a host-built -1e10 compact-mask bias, so the math is exactly the
reference's masked softmax.

Per-core pipeline per batch row (9 compact 128-token tiles as chunks of
512/512/128):
  - encT built by PE transpose-mode matmuls (128x128 tiles) into PSUM
    (bf16), evacuated to SBUF with a fused bf16->fp8e4m3 cast on DVE
    (ScalarE fp8 casts measured noisier on silicon; GpSimd has no PSUM
    port).
  - e_proj transposed (d on partitions) with fp8 DoubleRow matmuls
    (K=256 per instruction): lhsT = host-prequantized w_enc * 64 fp8,
    rhs = encT fp8 pairs. The 1/64 rescale and (h_proj + a_b) ride the
    tanh activation's scale/bias; tanh runs on [128, 1024] tiles (chunk
    pairs) to halve the per-op ScalarE init cost.
  - scores = v . tanh as columns: N=1 matmuls, th 128x128 slices
    stationary, v column moving -> scoresT in a [128, 9] PSUM tile
    (accumulation groups strictly sequential per column - start=True
    clears has_written bank-wide).
  - softmax unnormalized: compact-mask bias added to scoresT PSUM, Exp
    on ScalarE with accum_out row-sums, denominator closed by one
    cross-partition N=1 matmul; the 1/sum rescale lands once on the
    final weighted sum.
  - weighted sum as N=1 matmuls: lhsT = natural-layout gathered rows
    (bf16, unquantized - fp8 enc here would put ~4% noise on the
    output), rhs = p column.
"""

import numpy as np
from contextlib import ExitStack

B, S, ENC, DEC = 32, 2048, 1024, 1024
N_CORES = 8
BC = B // N_CORES   # batch rows per core
W_SCALE = 64.0      # fp8 weight pre-scale (avoids e4m3 subnormal range)
# padded compact-token count: Binomial(2048, 0.5) is 1024 +- 22.6, so 1152
# is a +5.7 sigma bound (seed-0 data maxes at 1062)
P_PAD = 1152
CHUNKS = (512, 512, 128)   # compact tokens per chunk (= 9 tiles of 128)


def build_bass_kernel(bc=BC, s=S, e_dim=ENC, d_dim=DEC, debug=False):
    import concourse.bass as bass
    import concourse.tile as tile
    from concourse import bacc, mybir

    f32 = mybir.dt.float32
    bf16 = mybir.dt.bfloat16
    fp8 = mybir.dt.float8e4
    i32 = mybir.dt.int32
    u16 = mybir.dt.uint16
    Tanh = mybir.ActivationFunctionType.Tanh
    Exp = mybir.ActivationFunctionType.Exp
    DR = mybir.MatmulPerfMode.DoubleRow

    n_et = e_dim // 128            # e 128-tiles (contraction for e_proj)
    n_dt = d_dim // 128            # d 128-tiles (e_proj output tiles)
    n_gt = P_PAD // 128            # compact s-tiles per batch row (9)
    n_kk = n_et // 2               # DoubleRow K=256 steps
    # (chunk, tile-within-chunk) for each global compact tile
    tile_map = []
    for c, csz in enumerate(CHUNKS):
        for jj in range(csz // 128):
            tile_map.append((c, jj))

    nc = bacc.Bacc("TRN2", target_bir_lowering=False, debug=debug)

    hs_h = nc.dram_tensor("hidden_states", [bc, d_dim], f32, kind="ExternalInput")
    enc_h = nc.dram_tensor("encoder_outputs", [bc, s, e_dim], f32, kind="ExternalInput")
    gidx_h = nc.dram_tensor("gidx", [bc, 128, n_gt], i32, kind="ExternalInput")
    cbias_h = nc.dram_tensor("cbias", [bc, 128, n_gt], f32, kind="ExternalInput")
    ab_h = nc.dram_tensor("a_b", [d_dim], f32, kind="ExternalInput")
    vw_h = nc.dram_tensor("v_w", [d_dim], f32, kind="ExternalInput")
    wenc8_h = nc.dram_tensor("w_enc_fp8", [128, n_et, d_dim], fp8, kind="ExternalInput")
    wd8_h = nc.dram_tensor("w_dec_fp8", [128, n_dt, d_dim], fp8, kind="ExternalInput")
    id_h = nc.dram_tensor("ident", [128, 128], bf16, kind="ExternalInput")
    out_h = nc.dram_tensor("out", [bc, e_dim], f32, kind="ExternalOutput")

    enc_flat = enc_h[:, :, :].rearrange("b s e -> (b s) e")

    with tile.TileContext(nc) as tc, ExitStack() as ctx:
        consts = ctx.enter_context(tc.tile_pool(name="consts", bufs=1))
        enc_pool = ctx.enter_context(tc.tile_pool(name="enc", bufs=10))
        encT_pool = ctx.enter_context(tc.tile_pool(name="encT", bufs=4))
        th_pool = ctx.enter_context(tc.tile_pool(name="tanh", bufs=10))
        p_pool = ctx.enter_context(tc.tile_pool(name="p", bufs=2))
        small_pool = ctx.enter_context(tc.tile_pool(name="small", bufs=6))
        outsb_pool = ctx.enter_context(tc.tile_pool(name="outsb", bufs=2))
        pe_psum = ctx.enter_context(tc.tile_pool(name="pe_psum", bufs=2, space="PSUM"))
        tr_psum = ctx.enter_context(tc.tile_pool(name="tr_psum", bufs=2, space="PSUM"))
        sc_psum = ctx.enter_context(tc.tile_pool(name="sc_psum", bufs=1, space="PSUM"))
        w_psum = ctx.enter_context(tc.tile_pool(name="w_psum", bufs=1, space="PSUM"))

        # ---------------- prologue DMAs (transfers serialize; this order
        # is the pipeline-fill critical path) ----------------
        gidx_sb = consts.tile([128, bc, n_gt], i32)
        nc.sync.dma_start(out=gidx_sb, in_=gidx_h[:, :, :].rearrange("b p g -> p b g"))

        cbias_sb = consts.tile([128, bc, n_gt], f32)
        nc.sync.dma_start(
            out=cbias_sb, in_=cbias_h[:, :, :].rearrange("b p g -> p b g")
        )

        id_sb = consts.tile([128, 128], bf16)
        nc.sync.dma_start(out=id_sb, in_=id_h[:, :])

        hs_bf = consts.tile([bc, d_dim], bf16)
        nc.gpsimd.dma_start(out=hs_bf, in_=hs_h[:, :])  # cast f32->bf16

        enc_chunks = {}
        state = {}

        def emit_gather(b, c):
            """Gather unmasked encoder rows for chunk (b, c): one
            128-index SWDGE call per 128-token tile (the silicon-
            validated gather shape), f32->bf16 cast in the DMA."""
            enc_c = enc_pool.tile([128, 4, e_dim], bf16, tag="enc")
            g0 = sum(cs // 128 for cs in CHUNKS[:c])
            for jj in range(CHUNKS[c] // 128):
                nc.gpsimd.indirect_dma_start(
                    out=enc_c[:, jj, :],
                    out_offset=None,
                    in_=enc_flat,
                    in_offset=bass.IndirectOffsetOnAxis(
                        ap=gidx_sb[:, b, g0 + jj : g0 + jj + 1], axis=0
                    ),
                )
            enc_chunks[(b, c)] = enc_c

        emit_gather(0, 0)
        emit_gather(0, 1)
        emit_gather(0, 2)

        # weights in kk-pair slices: the DMA device serves transfers in
        # arrival order, and page-sized pieces interleave with the
        # batch-0 gather stream instead of blocking it for 6us
        wenc8_sb = consts.tile([128, n_et, d_dim], fp8)
        for kk in range(n_kk):
            nc.sync.dma_start(
                out=wenc8_sb[:, 2 * kk : 2 * kk + 2, :],
                in_=wenc8_h[:, 2 * kk : 2 * kk + 2, :],
            )

        wd8_sb = consts.tile([128, n_dt, d_dim], fp8)
        for kk in range(n_kk):
            nc.sync.dma_start(
                out=wd8_sb[:, 2 * kk : 2 * kk + 2, :],
                in_=wd8_h[:, 2 * kk : 2 * kk + 2, :],
            )

        emit_gather(1, 0)
        emit_gather(1, 1)
        emit_gather(1, 2)
        v_sb = consts.tile([128, n_dt], bf16)
        nc.gpsimd.dma_start(out=v_sb, in_=vw_h[:].rearrange("(i p) -> p i", p=128))

        ab_sb = consts.tile([128, n_dt], f32)
        nc.sync.dma_start(out=ab_sb, in_=ab_h[:].rearrange("(i p) -> p i", p=128))

        ones_col = consts.tile([128, 1], f32)
        nc.vector.memset(ones_col, 1.0)
        ones_row = consts.tile([1, 128], f32)
        nc.vector.memset(ones_row, 1.0)
        ones4 = consts.tile([128, bc], f32)
        nc.vector.memset(ones4, 1.0)
        # a_b broadcast to (d-tile, b) layout: ab_rep[p, i, :] = a_b[128i+p]
        ab_rep = consts.tile([128, n_dt, bc], f32)
        for i in range(n_dt):
            nc.vector.tensor_scalar_mul(ab_rep[:, i, :], ones4, ab_sb[:, i : i + 1])

        # ---------------- h_proj (tiny; emitted via mid-hook inside the
        # first e_proj so the in-order PE queue isn't head-blocked while
        # w_dec_fp8 is still in flight) ----------------
        hb_sb = consts.tile([128, n_dt, bc], f32)

        hproj_state = {}

        def emit_hproj_a():
            # hiddenT (d on partitions) via K=bc transpose-by-matmul,
            # emitted in the prologue: PE and DVE are otherwise idle
            # waiting for the first gathers, and this keeps the fp8 cast
            # ahead of the evacuation backlog in the in-order DVE queue.
            # PSUM comes from tr_psum: pe_psum buffers hold un-evacuated
            # e_proj output whose tanh waits on hb -> using them here
            # would deadlock the PE queue.
            psum_h = tr_psum.tile([128, n_dt * bc], f32, tag="tr")
            for k in range(n_dt):
                nc.tensor.matmul(
                    psum_h[:, bc * k : bc * (k + 1)],
                    lhsT=hs_bf[:, 128 * k : 128 * (k + 1)],
                    rhs=id_sb[0:bc, 0:bc],
                    start=True,
                    stop=True,
                )
            hT8 = consts.tile([128, n_dt, bc], fp8)
            nc.vector.tensor_copy(hT8, psum_h)
            hproj_state["hT8"] = hT8

        def emit_hproj():
            hT8 = hproj_state["hT8"]
            # single-PSUM accumulation: per-i-block groups run strictly
            # sequentially in one bank (start=True clears has_written
            # bank-wide but leaves data; closed blocks are never
            # re-accumulated)
            psum_hp = tr_psum.tile([128, n_dt * bc], f32, tag="tr")
            for i in range(n_dt):
                for k in range(n_dt):
                    nc.tensor.matmul(
                        psum_hp[:, bc * i : bc * (i + 1)],
                        lhsT=wd8_sb[:, k, 128 * i : 128 * (i + 1)],
                        rhs=hT8[:, k, :],
                        start=(k == 0),
                        stop=(k == n_dt - 1),
                    )
            # hb = psum / W_SCALE + a_b (weights were pre-scaled *64)
            nc.vector.scalar_tensor_tensor(
                hb_sb.rearrange("p a b -> p (a b)"),
                psum_hp,
                1.0 / W_SCALE,
                ab_rep.rearrange("p a b -> p (a b)"),
                op0=mybir.AluOpType.mult,
                op1=mybir.AluOpType.add,
            )

        # ---------------- per-chunk stages ----------------

        def emit_transpose_j(b, c, j):
            """One 128-token tile of encT for chunk (b, c): 8 PE
            transposes (all e-tiles of tile j) into a PSUM bank + one
            cast-evacuation (ScalarE where it would otherwise idle, DVE
            steady; GpSimd has no PSUM port). Per-tile units mean a unit
            only waits on its own gather op."""
            if (b, c) not in state:
                state[(b, c)] = encT_pool.tile(
                    [128, n_et, 512], fp8, tag="encT", name="encT8"
                )
            encT8 = state[(b, c)]
            chunk = enc_chunks[(b, c)]
            tp = tr_psum.tile([128, n_et, 128], bf16, tag="tr", name="tp")
            for et in range(n_et):
                nc.tensor.transpose(
                    tp[:, et, :],
                    chunk[:, j, 128 * et : 128 * (et + 1)],
                    id_sb,
                )
            dst = encT8[:, :, 128 * j : 128 * (j + 1)]
            nc.vector.tensor_copy(dst, tp)

        def emit_transposes(b, c):
            for j in range(CHUNKS[c] // 128):
                emit_transpose_j(b, c, j)

        def emit_eproj_pair(b, mid_hook=None):
            # chunks 0+1 together: tanh runs on [128, 1024] tiles (one
            # ScalarE init per two chunks); the two 512-wide matmul
            # groups land in the two banks of a 2-bank PSUM tile.
            eTa = state.pop((b, 0))
            eTb = state.pop((b, 1))
            state[("sc", b)] = sc_psum.tile([128, n_gt], f32, tag="sc", name="psc")
            if mid_hook is not None:
                mid_hook()
                mid_hook = None
            ths = []
            for i in range(n_dt):
                pe = pe_psum.tile([128, 2, 512], f32, tag="pe")
                for half, eT in ((0, eTa), (1, eTb)):
                    for kk in range(n_kk):
                        nc.tensor.matmul(
                            pe[:, half, :],
                            lhsT=wenc8_sb[
                                :, 2 * kk : 2 * kk + 2, 128 * i : 128 * (i + 1)
                            ],
                            rhs=eT[:, 2 * kk : 2 * kk + 2, :],
                            start=(kk == 0),
                            stop=(kk == n_kk - 1),
                            perf_mode=DR,
                        )
                th = th_pool.tile([128, 2, 512], bf16, tag="tanh")
                nc.scalar.activation(
                    th.rearrange("p a b -> p (a b)"),
                    pe.rearrange("p a b -> p (a b)"),
                    Tanh,
                    bias=hb_sb[:, i, b : b + 1],
                    scale=1.0 / W_SCALE,
                )
                ths.append(th)
            state[("th", b)] = ths

        def emit_eproj_tail(b):
            # chunk 2: single 128-token tile
            eT = state.pop((b, 2))
            ths = []
            for i in range(n_dt):
                pe = pe_psum.tile([128, 2, 512], f32, tag="pe")
                for kk in range(n_kk):
                    nc.tensor.matmul(
                        pe[:, 0, 0:128],
                        lhsT=wenc8_sb[:, 2 * kk : 2 * kk + 2, 128 * i : 128 * (i + 1)],
                        rhs=eT[:, 2 * kk : 2 * kk + 2, 0:128],
                        start=(kk == 0),
                        stop=(kk == n_kk - 1),
                        perf_mode=DR,
                    )
                th = th_pool.tile([128, 128], bf16, tag="ttail")
                nc.scalar.activation(
                    th, pe[:, 0, 0:128], Tanh, bias=hb_sb[:, i, b : b + 1],
                    scale=1.0 / W_SCALE,
                )
                ths.append(th)
            state[("tht", b)] = ths

        scores_done = {}

        def emit_scores_col(b, col):
            # Column-outer, i-inner: accumulation groups in the scoresT
            # bank must be strictly sequential (start=True clears
            # has_written for the WHOLE bank).
            ths = state[("th", b)]
            psum_sc = state[("sc", b)]
            half, jj = divmod(col, 4)
            for i in range(n_dt):
                nc.tensor.matmul(
                    psum_sc[:, col : col + 1],
                    lhsT=ths[i][:, half, 128 * jj : 128 * (jj + 1)],
                    rhs=v_sb[:, i : i + 1],
                    start=(i == 0),
                    stop=(i == n_dt - 1),
                )
            scores_done[b] = scores_done.get(b, 0) + 1

        def emit_scores_pair(b):
            for col in range(scores_done.get(b, 0), 8):
                emit_scores_col(b, col)
            state.pop(("th", b))

        def emit_scores_tail(b):
            ths = state.pop(("tht", b))
            psum_sc = state[("sc", b)]
            for i in range(n_dt):
                nc.tensor.matmul(
                    psum_sc[:, 8:9],
                    lhsT=ths[i],
                    rhs=v_sb[:, i : i + 1],
                    start=(i == 0),
                    stop=(i == n_dt - 1),
                )

        def emit_softmax_a(b):
            """Compact-mask bias + exp with fused row-sums (DVE+ScalarE)."""
            psum_sc = state.pop(("sc", b))
            nc.vector.tensor_add(psum_sc, psum_sc, cbias_sb[:, b, :])
            p_bf = p_pool.tile([128, n_gt], bf16, tag="p")
            rowsum = small_pool.tile([128, 1], f32, tag="rowsum")
            nc.scalar.activation(
                p_bf, psum_sc, Exp, bias=0.0, scale=1.0, accum_out=rowsum
            )
            state[("p", b)] = p_bf
            state[("rowsum", b)] = rowsum

        def emit_ssum_recip(b):
            rowsum = state.pop(("rowsum", b))
            ssum = w_psum.tile([1, 1], f32, tag="w")
            nc.tensor.matmul(ssum, lhsT=rowsum, rhs=ones_col, start=True, stop=True)
            rsum = small_pool.tile([1, 1], f32, tag="rsum")
            nc.vector.reciprocal(rsum, ssum)
            state[("rsum", b)] = rsum

        def emit_weighted(b):
            p_bf = state.pop(("p", b))
            rsum = state.pop(("rsum", b))
            rbc_ps = w_psum.tile([128, 1], f32, tag="w")
            nc.tensor.matmul(rbc_ps, lhsT=ones_row, rhs=rsum, start=True, stop=True)
            rbc = small_pool.tile([128, 1], f32, tag="rbc")
            nc.vector.tensor_copy(rbc, rbc_ps)
            w_ps = w_psum.tile([128, n_dt], f32, tag="w")
            for i in range(n_et):
                for g, (c, jj) in enumerate(tile_map):
                    nc.tensor.matmul(
                        w_ps[:, i : i + 1],
                        lhsT=enc_chunks[(b, c)][:, jj, 128 * i : 128 * (i + 1)],
                        rhs=p_bf[:, g : g + 1],
                        start=(g == 0),
                        stop=(g == n_gt - 1),
                    )
            for c in range(len(CHUNKS)):
                del enc_chunks[(b, c)]
            out_sb = outsb_pool.tile([128, n_et], f32, tag="outsb")
            nc.vector.tensor_scalar_mul(out_sb, w_ps, rbc[:, 0:1])
            nc.sync.dma_start(
                out=out_h[b, :].rearrange("(i p) -> p i", p=128), in_=out_sb
            )

        # ---------------- schedule ----------------
        # Two sub-stages per batch row: A(b) = chunks 0+1 e_proj, B(b) =
        # tail e_proj. Transposes run one sub-stage ahead of their
        # e_proj, scores one sub-stage behind, so the in-order PE queue
        # never blocks on ScalarE/DVE results.
        emit_hproj_a()
        emit_transposes(0, 0)
        emit_transposes(0, 1)
        for b in range(bc):
            # --- sub-stage A(b) ---
            if b + 2 < bc:
                emit_gather(b + 2, 0)
                emit_gather(b + 2, 1)
            emit_eproj_pair(b, mid_hook=emit_hproj if b == 0 else None)
            emit_transposes(b, 2)
            if b > 0:
                emit_scores_tail(b - 1)
                emit_softmax_a(b - 1)
            # --- sub-stage B(b) ---
            if b + 2 < bc:
                emit_gather(b + 2, 2)
            emit_eproj_tail(b)
            if b + 1 < bc:
                emit_transposes(b + 1, 0)
                emit_transposes(b + 1, 1)
            emit_scores_pair(b)
            if b > 0:
                emit_ssum_recip(b - 1)
                emit_weighted(b - 1)
        emit_scores_tail(bc - 1)
        emit_softmax_a(bc - 1)
        emit_ssum_recip(bc - 1)
        emit_weighted(bc - 1)

    nc.compile()
    return nc


_CACHE = {}


def _prep_weights(a_w):
    """Host-side weight repack: w_enc and w_dec scaled by 64 and
    quantized to fp8e4m3 in (p, k, d) layout matching the stationary-
    operand slices (DoubleRow pairs for w_enc)."""
    import ml_dtypes

    def pack(w):
        w = (np.asarray(w, dtype=np.float32) * W_SCALE).reshape(-1, 128, DEC)
        return np.ascontiguousarray(w.transpose(1, 0, 2)).astype(
            ml_dtypes.float8_e4m3
        )

    return pack(a_w[DEC:]), pack(a_w[:DEC])


def _prep_indices(masks):
    """Per-row unmasked token indices (padded to P_PAD with row 0 of the
    same batch row - its lanes are killed by cbias) and the compact-mask
    bias, both in column-major (p, g) tile layout."""
    bc = masks.shape[0]
    gidx = np.zeros((bc, P_PAD), dtype=np.int32)
    cbias = np.full((bc, P_PAD), -1e10, dtype=np.float32)
    for b in range(bc):
        idx = np.nonzero(masks[b])[0].astype(np.int32)
        cnt = len(idx)
        assert cnt <= P_PAD, f"unmasked count {cnt} exceeds P_PAD={P_PAD}"
        gidx[b, :cnt] = b * S + idx
        gidx[b, cnt:] = b * S
        cbias[b, :cnt] = 0.0
    # (b, tile*128 + p) -> (b, p, tile)
    gidx = np.ascontiguousarray(gidx.reshape(bc, P_PAD // 128, 128).transpose(0, 2, 1))
    cbias = np.ascontiguousarray(
        cbias.reshape(bc, P_PAD // 128, 128).transpose(0, 2, 1)
    )
    return gidx, cbias


def kernel(hidden_states, encoder_outputs, encoder_masks, a_w, a_b, v_w):
    import ml_dtypes
    from concourse.bass_utils import run_bass_kernel_spmd

    if "nc" not in _CACHE:
        _CACHE["nc"] = build_bass_kernel()
    nc = _CACHE["nc"]

    hidden_states = np.asarray(hidden_states, dtype=np.float32)
    encoder_outputs = np.asarray(encoder_outputs, dtype=np.float32)
    encoder_masks = np.asarray(encoder_masks, dtype=np.int32)
    a_w = np.ascontiguousarray(np.asarray(a_w, dtype=np.float32))
    a_b = np.ascontiguousarray(np.asarray(a_b, dtype=np.float32))
    v_w = np.ascontiguousarray(np.asarray(v_w, dtype=np.float32))
    ident = np.eye(128, dtype=ml_dtypes.bfloat16)
    wenc8, wd8 = _prep_weights(a_w)

    in_maps = []
    for c in range(N_CORES):
        sl = slice(c * BC, (c + 1) * BC)
        gidx, cbias = _prep_indices(encoder_masks[sl])
        in_maps.append(
            {
                "hidden_states": np.ascontiguousarray(hidden_states[sl]),
                "encoder_outputs": np.ascontiguousarray(encoder_outputs[sl]),
                "gidx": gidx,
                "cbias": cbias,
                "a_b": a_b,
                "v_w": v_w,
                "w_enc_fp8": wenc8,
                "w_dec_fp8": wd8,
                "ident": ident,
            }
        )

    global _LAST_IN_MAPS
    _LAST_IN_MAPS = in_maps
    res = run_bass_kernel_spmd(nc, in_maps, core_ids=list(range(N_CORES)))
    out = np.concatenate([r["out"] for r in res.results], axis=0)
    return out.astype(np.float32)


_LAST_IN_MAPS = None


# revision 37
# speedup vs baseline: 3.9698x; 1.0067x over previous
"""Bahdanau-style attention kernel for Trainium2 (8 NeuronCores, SPMD).

Math (per batch row b):
    h_proj = hidden @ a_w[:DEC]                       (DEC,)
    e_proj[s, :] = enc[s, :] @ a_w[DEC:]              (S, DEC)
    energy = tanh(e_proj + h_proj + a_b)              (S, DEC)
    scores = energy @ v_w                             (S,)
    scores = where(mask == 0, -1e10, scores)
    attn = softmax(scores)                            (S,)
    out = attn @ enc                                  (ENC,)

Sharding: data-parallel over batch (32 rows -> 4 rows on each of 8 cores);
weights replicated.

Masked tokens get attn == 0 exactly, so only the unmasked rows (~half;
Binomial(2048, .5), padded to P_PAD=1152 = +5.7 sigma) contribute to any
output. The host computes each row's unmasked-index list (cheap metadata,
<0.01% of the FLOPs - the kernel-side equivalent was measured
descriptor-bound on Q7) and the device gathers just those encoder rows
with indirect SWDGE DMAs (fp32->bf16 cast in flight, one 128-index call
per tile - the silicon-validated gather shape). Pad lanes are killed by
a host-built -1e10 compact-mask bias, so the math is exactly the
reference's masked softmax.

Per-core pipeline per batch row (9 compact 128-token tiles as chunks of
512/512/128):
  - encT built by PE transpose-mode matmuls (128x128 tiles) into PSUM
    (bf16), evacuated to SBUF with a fused bf16->fp8e4m3 cast on DVE
    (ScalarE fp8 casts measured noisier on silicon; GpSimd has no PSUM
    port).
  - e_proj transposed (d on partitions) with fp8 DoubleRow matmuls
    (K=256 per instruction): lhsT = host-prequantized w_enc * 64 fp8,
    rhs = encT fp8 pairs. The 1/64 rescale and (h_proj + a_b) ride the
    tanh activation's scale/bias; tanh runs on [128, 1024] tiles (chunk
    pairs) to halve the per-op ScalarE init cost.
  - scores = v . tanh as columns: N=1 matmuls, th 128x128 slices
    stationary, v column moving -> scoresT in a [128, 9] PSUM tile
    (accumulation groups strictly sequential per column - start=True
    clears has_written bank-wide).
  - softmax unnormalized: compact-mask bias added to scoresT PSUM, Exp
    on ScalarE with accum_out row-sums, denominator closed by one
    cross-partition N=1 matmul; the 1/sum rescale lands once on the
    final weighted sum.
  - weighted sum as N=1 matmuls: lhsT = natural-layout gathered rows
    (bf16, unquantized - fp8 enc here would put ~4% noise on the
    output), rhs = p column.
"""

import numpy as np
from contextlib import ExitStack

B, S, ENC, DEC = 32, 2048, 1024, 1024
N_CORES = 8
BC = B // N_CORES   # batch rows per core
W_SCALE = 64.0      # fp8 weight pre-scale (avoids e4m3 subnormal range)
# padded compact-token count: Binomial(2048, 0.5) is 1024 +- 22.6, so 1152
# is a +5.7 sigma bound (seed-0 data maxes at 1062)
P_PAD = 1152
CHUNKS = (512, 512, 128)   # compact tokens per chunk (= 9 tiles of 128)


def build_bass_kernel(bc=BC, s=S, e_dim=ENC, d_dim=DEC, debug=False):
    import concourse.bass as bass
    import concourse.tile as tile
    from concourse import bacc, mybir

    f32 = mybir.dt.float32
    bf16 = mybir.dt.bfloat16
    fp8 = mybir.dt.float8e4
    i32 = mybir.dt.int32
    u16 = mybir.dt.uint16
    Tanh = mybir.ActivationFunctionType.Tanh
    Exp = mybir.ActivationFunctionType.Exp
    DR = mybir.MatmulPerfMode.DoubleRow

    n_et = e_dim // 128            # e 128-tiles (contraction for e_proj)
    n_dt = d_dim // 128            # d 128-tiles (e_proj output tiles)
    n_gt = P_PAD // 128            # compact s-tiles per batch row (9)
    n_kk = n_et // 2               # DoubleRow K=256 steps
    # (chunk, tile-within-chunk) for each global compact tile
    tile_map = []
    for c, csz in enumerate(CHUNKS):
        for jj in range(csz // 128):
            tile_map.append((c, jj))

    nc = bacc.Bacc("TRN2", target_bir_lowering=False, debug=debug)

    hs_h = nc.dram_tensor("hidden_states", [bc, d_dim], f32, kind="ExternalInput")
    enc_h = nc.dram_tensor("encoder_outputs", [bc, s, e_dim], f32, kind="ExternalInput")
    gidx_h = nc.dram_tensor("gidx", [bc, 128, n_gt], i32, kind="ExternalInput")
    cbias_h = nc.dram_tensor("cbias", [bc, 128, n_gt], f32, kind="ExternalInput")
    ab_h = nc.dram_tensor("a_b", [d_dim], f32, kind="ExternalInput")
    vw_h = nc.dram_tensor("v_w", [d_dim], f32, kind="ExternalInput")
    wenc8_h = nc.dram_tensor("w_enc_fp8", [128, n_et, d_dim], fp8, kind="ExternalInput")
    wd8_h = nc.dram_tensor("w_dec_fp8", [128, n_dt, d_dim], fp8, kind="ExternalInput")
    id_h = nc.dram_tensor("ident", [128, 128], bf16, kind="ExternalInput")
    out_h = nc.dram_tensor("out", [bc, e_dim], f32, kind="ExternalOutput")

    enc_flat = enc_h[:, :, :].rearrange("b s e -> (b s) e")

    with tile.TileContext(nc) as tc, ExitStack() as ctx:
        consts = ctx.enter_context(tc.tile_pool(name="consts", bufs=1))
        enc_pool = ctx.enter_context(tc.tile_pool(name="enc", bufs=10))
        encT_pool = ctx.enter_context(tc.tile_pool(name="encT", bufs=4))
        th_pool = ctx.enter_context(tc.tile_pool(name="tanh", bufs=10))
        p_pool = ctx.enter_context(tc.tile_pool(name="p", bufs=2))
        small_pool = ctx.enter_context(tc.tile_pool(name="small", bufs=6))
        outsb_pool = ctx.enter_context(tc.tile_pool(name="outsb", bufs=2))
        pe_psum = ctx.enter_context(tc.tile_pool(name="pe_psum", bufs=2, space="PSUM"))
        tr_psum = ctx.enter_context(tc.tile_pool(name="tr_psum", bufs=2, space="PSUM"))
        sc_psum = ctx.enter_context(tc.tile_pool(name="sc_psum", bufs=1, space="PSUM"))
        w_psum = ctx.enter_context(tc.tile_pool(name="w_psum", bufs=1, space="PSUM"))

        # ---------------- prologue DMAs (transfers serialize; this order
        # is the pipeline-fill critical path) ----------------
        gidx_sb = consts.tile([128, bc, n_gt], i32)
        nc.sync.dma_start(out=gidx_sb, in_=gidx_h[:, :, :].rearrange("b p g -> p b g"))

        cbias_sb = consts.tile([128, bc, n_gt], f32)
        nc.sync.dma_start(
            out=cbias_sb, in_=cbias_h[:, :, :].rearrange("b p g -> p b g")
        )

        id_sb = consts.tile([128, 128], bf16)
        nc.sync.dma_start(out=id_sb, in_=id_h[:, :])

        hs_bf = consts.tile([bc, d_dim], bf16)
        nc.gpsimd.dma_start(out=hs_bf, in_=hs_h[:, :])  # cast f32->bf16

        enc_chunks = {}
        state = {}

        def emit_gather(b, c):
            """Gather unmasked encoder rows for chunk (b, c): one
            128-index SWDGE call per 128-token tile (the silicon-
            validated gather shape), f32->bf16 cast in the DMA."""
            enc_c = enc_pool.tile([128, 4, e_dim], bf16, tag="enc")
            g0 = sum(cs // 128 for cs in CHUNKS[:c])
            for jj in range(CHUNKS[c] // 128):
                nc.gpsimd.indirect_dma_start(
                    out=enc_c[:, jj, :],
                    out_offset=None,
                    in_=enc_flat,
                    in_offset=bass.IndirectOffsetOnAxis(
                        ap=gidx_sb[:, b, g0 + jj : g0 + jj + 1], axis=0
                    ),
                )
            enc_chunks[(b, c)] = enc_c

        emit_gather(0, 0)
        emit_gather(0, 1)
        emit_gather(0, 2)

        # weights in kk-pair slices: the DMA device serves transfers in
        # arrival order, and page-sized pieces interleave with the
        # batch-0 gather stream instead of blocking it for 6us
        wenc8_sb = consts.tile([128, n_et, d_dim], fp8)
        for kk in range(n_kk):
            nc.sync.dma_start(
                out=wenc8_sb[:, 2 * kk : 2 * kk + 2, :],
                in_=wenc8_h[:, 2 * kk : 2 * kk + 2, :],
            )

        wd8_sb = consts.tile([128, n_dt, d_dim], fp8)
        for kk in range(n_kk):
            nc.sync.dma_start(
                out=wd8_sb[:, 2 * kk : 2 * kk + 2, :],
                in_=wd8_h[:, 2 * kk : 2 * kk + 2, :],
            )

        emit_gather(1, 0)
        emit_gather(1, 1)
        emit_gather(1, 2)
        v_sb = consts.tile([128, n_dt], bf16)
        nc.gpsimd.dma_start(out=v_sb, in_=vw_h[:].rearrange("(i p) -> p i", p=128))

        ab_sb = consts.tile([128, n_dt], f32)
        nc.sync.dma_start(out=ab_sb, in_=ab_h[:].rearrange("(i p) -> p i", p=128))

        ones_col = consts.tile([128, 1], f32)
        nc.vector.memset(ones_col, 1.0)
        ones_row = consts.tile([1, 128], f32)
        nc.vector.memset(ones_row, 1.0)
        ones4 = consts.tile([128, bc], f32)
        nc.vector.memset(ones4, 1.0)
        # a_b broadcast to (d-tile, b) layout: ab_rep[p, i, :] = a_b[128i+p]
        ab_rep = consts.tile([128, n_dt, bc], f32)
        for i in range(n_dt):
            nc.vector.tensor_scalar_mul(ab_rep[:, i, :], ones4, ab_sb[:, i : i + 1])

        # ---------------- h_proj (tiny; emitted via mid-hook inside the
        # first e_proj so the in-order PE queue isn't head-blocked while
        # w_dec_fp8 is still in flight) ----------------
        hb_sb = consts.tile([128, n_dt, bc], f32)

        hproj_state = {}

        def emit_hproj_a():
            # hiddenT (d on partitions) via K=bc transpose-by-matmul,
            # emitted in the prologue: PE and DVE are otherwise idle
            # waiting for the first gathers, and this keeps the fp8 cast
            # ahead of the evacuation backlog in the in-order DVE queue.
            # PSUM comes from tr_psum: pe_psum buffers hold un-evacuated
            # e_proj output whose tanh waits on hb -> using them here
            # would deadlock the PE queue.
            psum_h = tr_psum.tile([128, n_dt * bc], f32, tag="tr")
            for k in range(n_dt):
                nc.tensor.matmul(
                    psum_h[:, bc * k : bc * (k + 1)],
                    lhsT=hs_bf[:, 128 * k : 128 * (k + 1)],
                    rhs=id_sb[0:bc, 0:bc],
                    start=True,
                    stop=True,
                )
            hT8 = consts.tile([128, n_dt, bc], fp8)
            nc.vector.tensor_copy(hT8, psum_h)
            hproj_state["hT8"] = hT8

        def emit_hproj():
            hT8 = hproj_state["hT8"]
            # single-PSUM accumulation: per-i-block groups run strictly
            # sequentially in one bank (start=True clears has_written
            # bank-wide but leaves data; closed blocks are never
            # re-accumulated)
            psum_hp = tr_psum.tile([128, n_dt * bc], f32, tag="tr")
            for i in range(n_dt):
                for k in range(n_dt):
                    nc.tensor.matmul(
                        psum_hp[:, bc * i : bc * (i + 1)],
                        lhsT=wd8_sb[:, k, 128 * i : 128 * (i + 1)],
                        rhs=hT8[:, k, :],
                        start=(k == 0),
                        stop=(k == n_dt - 1),
                    )
            # hb = psum / W_SCALE + a_b (weights were pre-scaled *64)
            nc.vector.scalar_tensor_tensor(
                hb_sb.rearrange("p a b -> p (a b)"),
                psum_hp,
                1.0 / W_SCALE,
                ab_rep.rearrange("p a b -> p (a b)"),
                op0=mybir.AluOpType.mult,
                op1=mybir.AluOpType.add,
            )

        # ---------------- per-chunk stages ----------------

        def emit_transpose_j(b, c, j):
            """One 128-token tile of encT for chunk (b, c): 8 PE
            transposes (all e-tiles of tile j) into a PSUM bank + one
            cast-evacuation (ScalarE where it would otherwise idle, DVE
            steady; GpSimd has no PSUM port). Per-tile units mean a unit
            only waits on its own gather op."""
            if (b, c) not in state:
                state[(b, c)] = encT_pool.tile(
                    [128, n_et, 512], fp8, tag="encT", name="encT8"
                )
            encT8 = state[(b, c)]
            chunk = enc_chunks[(b, c)]
            tp = tr_psum.tile([128, n_et, 128], bf16, tag="tr", name="tp")
            for et in range(n_et):
                nc.tensor.transpose(
                    tp[:, et, :],
                    chunk[:, j, 128 * et : 128 * (et + 1)],
                    id_sb,
                )
            dst = encT8[:, :, 128 * j : 128 * (j + 1)]
            nc.vector.tensor_copy(dst, tp)

        def emit_transposes(b, c):
            for j in range(CHUNKS[c] // 128):
                emit_transpose_j(b, c, j)

        def emit_eproj_pair(b, mid_hook=None):
            # chunks 0+1 together: tanh runs on [128, 1024] tiles (one
            # ScalarE init per two chunks); the two 512-wide matmul
            # groups land in the two banks of a 2-bank PSUM tile.
            eTa = state.pop((b, 0))
            eTb = state.pop((b, 1))
            state[("sc", b)] = sc_psum.tile([128, n_gt], f32, tag="sc", name="psc")
            if mid_hook is not None:
                mid_hook()
                mid_hook = None
            ths = []
            for i in range(n_dt):
                pe = pe_psum.tile([128, 2, 512], f32, tag="pe")
                for half, eT in ((0, eTa), (1, eTb)):
                    for kk in range(n_kk):
                        nc.tensor.matmul(
                            pe[:, half, :],
                            lhsT=wenc8_sb[
                                :, 2 * kk : 2 * kk + 2, 128 * i : 128 * (i + 1)
                            ],
                            rhs=eT[:, 2 * kk : 2 * kk + 2, :],
                            start=(kk == 0),
                            stop=(kk == n_kk - 1),
                            perf_mode=DR,
                        )
                th = th_pool.tile([128, 2, 512], bf16, tag="tanh")
                nc.scalar.activation(
                    th.rearrange("p a b -> p (a b)"),
                    pe.rearrange("p a b -> p (a b)"),
                    Tanh,
                    bias=hb_sb[:, i, b : b + 1],
                    scale=1.0 / W_SCALE,
                )
                ths.append(th)
            state[("th", b)] = ths

        def emit_eproj_tail(b):
            # chunk 2: single 128-token tile
            eT = state.pop((b, 2))
            ths = []
            for i in range(n_dt):
                pe = pe_psum.tile([128, 2, 512], f32, tag="pe")
                for kk in range(n_kk):
                    nc.tensor.matmul(
                        pe[:, 0, 0:128],
                        lhsT=wenc8_sb[:, 2 * kk : 2 * kk + 2, 128 * i : 128 * (i + 1)],
                        rhs=eT[:, 2 * kk : 2 * kk + 2, 0:128],
                        start=(kk == 0),
                        stop=(kk == n_kk - 1),
                        perf_mode=DR,
                    )
                th = th_pool.tile([128, 128], bf16, tag="ttail")
                nc.scalar.activation(
                    th, pe[:, 0, 0:128], Tanh, bias=hb_sb[:, i, b : b + 1],
                    scale=1.0 / W_SCALE,
                )
                ths.append(th)
            state[("tht", b)] = ths

        scores_done = {}

        def emit_scores_col(b, col):
            # Column-outer, i-inner: accumulation groups in the scoresT
            # bank must be strictly sequential (start=True clears
            # has_written for the WHOLE bank).
            ths = state[("th", b)]
            psum_sc = state[("sc", b)]
            half, jj = divmod(col, 4)
            for i in range(n_dt):
                nc.tensor.matmul(
                    psum_sc[:, col : col + 1],
                    lhsT=ths[i][:, half, 128 * jj : 128 * (jj + 1)],
                    rhs=v_sb[:, i : i + 1],
                    start=(i == 0),
                    stop=(i == n_dt - 1),
                )
            scores_done[b] = scores_done.get(b, 0) + 1

        def emit_scores_pair(b):
            for col in range(scores_done.get(b, 0), 8):
                emit_scores_col(b, col)
            state.pop(("th", b))

        def emit_scores_tail(b):
            ths = state.pop(("tht", b))
            psum_sc = state[("sc", b)]
            for i in range(n_dt):
                nc.tensor.matmul(
                    psum_sc[:, 8:9],
                    lhsT=ths[i],
                    rhs=v_sb[:, i : i + 1],
                    start=(i == 0),
                    stop=(i == n_dt - 1),
                )

        def emit_softmax_a(b):
            """Compact-mask bias + exp with fused row-sums (DVE+ScalarE)."""
            psum_sc = state.pop(("sc", b))
            nc.vector.tensor_add(psum_sc, psum_sc, cbias_sb[:, b, :])
            p_bf = p_pool.tile([128, n_gt], bf16, tag="p")
            rowsum = small_pool.tile([128, 1], f32, tag="rowsum")
            nc.scalar.activation(
                p_bf, psum_sc, Exp, bias=0.0, scale=1.0, accum_out=rowsum
            )
            state[("p", b)] = p_bf
            state[("rowsum", b)] = rowsum

        def emit_ssum_recip(b):
            rowsum = state.pop(("rowsum", b))
            ssum = w_psum.tile([1, 1], f32, tag="w")
            nc.tensor.matmul(ssum, lhsT=rowsum, rhs=ones_col, start=True, stop=True)
            rsum = small_pool.tile([1, 1], f32, tag="rsum")
            nc.vector.reciprocal(rsum, ssum)
            state[("rsum", b)] = rsum

        def emit_weighted(b):
            p_bf = state.pop(("p", b))
            rsum = state.pop(("rsum", b))
            rbc_ps = w_psum.tile([128, 1], f32, tag="w")
            nc.tensor.matmul(rbc_ps, lhsT=ones_row, rhs=rsum, start=True, stop=True)
            rbc = small_pool.tile([128, 1], f32, tag="rbc")
            nc.vector.tensor_copy(rbc, rbc_ps)
            w_ps = w_psum.tile([128, n_dt], f32, tag="w")
            for i in range(n_et):
                for g, (c, jj) in enumerate(tile_map):
                    nc.tensor.matmul(
                        w_ps[:, i : i + 1],
                        lhsT=enc_chunks[(b, c)][:, jj, 128 * i : 128 * (i + 1)],
                        rhs=p_bf[:, g : g + 1],
                        start=(g == 0),
                        stop=(g == n_gt - 1),
                    )
            for c in range(len(CHUNKS)):
                del enc_chunks[(b, c)]
            out_sb = outsb_pool.tile([128, n_et], f32, tag="outsb")
            nc.vector.tensor_scalar_mul(out_sb, w_ps, rbc[:, 0:1])
            nc.sync.dma_start(
                out=out_h[b, :].rearrange("(i p) -> p i", p=128), in_=out_sb
            )

        # ---------------- schedule ----------------
        # Two sub-stages per batch row: A(b) = chunks 0+1 e_proj, B(b) =
        # tail e_proj. Transposes run one sub-stage ahead of their
        # e_proj, scores one sub-stage behind, so the in-order PE queue
        # never blocks on ScalarE/DVE results.
        emit_hproj_a()
        emit_transposes(0, 0)
        emit_transposes(0, 1)
        for b in range(bc):
            # --- sub-stage A(b) ---
            if b + 2 < bc:
                emit_gather(b + 2, 0)
                emit_gather(b + 2, 1)
            if b > 0:
                emit_scores_tail(b - 1)
                emit_softmax_a(b - 1)
            emit_eproj_pair(b, mid_hook=emit_hproj if b == 0 else None)
            emit_transposes(b, 2)
            # --- sub-stage B(b) ---
            if b + 2 < bc:
                emit_gather(b + 2, 2)
            emit_eproj_tail(b)
            if b + 1 < bc:
                emit_transposes(b + 1, 0)
                emit_transposes(b + 1, 1)
            emit_scores_pair(b)
            if b > 0:
                emit_ssum_recip(b - 1)
                emit_weighted(b - 1)
        emit_scores_tail(bc - 1)
        emit_softmax_a(bc - 1)
        emit_ssum_recip(bc - 1)
        emit_weighted(bc - 1)

    nc.compile()
    return nc


_CACHE = {}


def _prep_weights(a_w):
    """Host-side weight repack: w_enc and w_dec scaled by 64 and
    quantized to fp8e4m3 in (p, k, d) layout matching the stationary-
    operand slices (DoubleRow pairs for w_enc)."""
    import ml_dtypes

    def pack(w):
        w = (np.asarray(w, dtype=np.float32) * W_SCALE).reshape(-1, 128, DEC)
        return np.ascontiguousarray(w.transpose(1, 0, 2)).astype(
            ml_dtypes.float8_e4m3
        )

    return pack(a_w[DEC:]), pack(a_w[:DEC])


def _prep_indices(masks):
    """Per-row unmasked token indices (padded to P_PAD with row 0 of the
    same batch row - its lanes are killed by cbias) and the compact-mask
    bias, both in column-major (p, g) tile layout."""
    bc = masks.shape[0]
    gidx = np.zeros((bc, P_PAD), dtype=np.int32)
    cbias = np.full((bc, P_PAD), -1e10, dtype=np.float32)
    for b in range(bc):
        idx = np.nonzero(masks[b])[0].astype(np.int32)
        cnt = len(idx)
        assert cnt <= P_PAD, f"unmasked count {cnt} exceeds P_PAD={P_PAD}"
        gidx[b, :cnt] = b * S + idx
        gidx[b, cnt:] = b * S
        cbias[b, :cnt] = 0.0
    # (b, tile*128 + p) -> (b, p, tile)
    gidx = np.ascontiguousarray(gidx.reshape(bc, P_PAD // 128, 128).transpose(0, 2, 1))
    cbias = np.ascontiguousarray(
        cbias.reshape(bc, P_PAD // 128, 128).transpose(0, 2, 1)
    )
    return gidx, cbias


def kernel(hidden_states, encoder_outputs, encoder_masks, a_w, a_b, v_w):
    import ml_dtypes
    from concourse.bass_utils import run_bass_kernel_spmd

    if "nc" not in _CACHE:
        _CACHE["nc"] = build_bass_kernel()
    nc = _CACHE["nc"]

    hidden_states = np.asarray(hidden_states, dtype=np.float32)
    encoder_outputs = np.asarray(encoder_outputs, dtype=np.float32)
    encoder_masks = np.asarray(encoder_masks, dtype=np.int32)
    a_w = np.ascontiguousarray(np.asarray(a_w, dtype=np.float32))
    a_b = np.ascontiguousarray(np.asarray(a_b, dtype=np.float32))
    v_w = np.ascontiguousarray(np.asarray(v_w, dtype=np.float32))
    ident = np.eye(128, dtype=ml_dtypes.bfloat16)
    wenc8, wd8 = _prep_weights(a_w)

    in_maps = []
    for c in range(N_CORES):
        sl = slice(c * BC, (c + 1) * BC)
        gidx, cbias = _prep_indices(encoder_masks[sl])
        in_maps.append(
            {
                "hidden_states": np.ascontiguousarray(hidden_states[sl]),
                "encoder_outputs": np.ascontiguousarray(encoder_outputs[sl]),
                "gidx": gidx,
                "cbias": cbias,
                "a_b": a_b,
                "v_w": v_w,
                "w_enc_fp8": wenc8,
                "w_dec_fp8": wd8,
                "ident": ident,
            }
        )

    global _LAST_IN_MAPS
    _LAST_IN_MAPS = in_maps
    res = run_bass_kernel_spmd(nc, in_maps, core_ids=list(range(N_CORES)))
    out = np.concatenate([r["out"] for r in res.results], axis=0)
    return out.astype(np.float32)


_LAST_IN_MAPS = None


# revision 42
# speedup vs baseline: 3.9830x; 1.0033x over previous
"""Bahdanau-style attention kernel for Trainium2 (8 NeuronCores, SPMD).

Math (per batch row b):
    h_proj = hidden @ a_w[:DEC]                       (DEC,)
    e_proj[s, :] = enc[s, :] @ a_w[DEC:]              (S, DEC)
    energy = tanh(e_proj + h_proj + a_b)              (S, DEC)
    scores = energy @ v_w                             (S,)
    scores = where(mask == 0, -1e10, scores)
    attn = softmax(scores)                            (S,)
    out = attn @ enc                                  (ENC,)

Sharding: data-parallel over batch (32 rows -> 4 rows on each of 8 cores);
weights replicated.

Masked tokens get attn == 0 exactly, so only the unmasked rows (~half;
Binomial(2048, .5), padded to P_PAD=1152 = +5.7 sigma) contribute to any
output. The host computes each row's unmasked-index list (cheap metadata,
<0.01% of the FLOPs - the kernel-side equivalent was measured
descriptor-bound on Q7) and the device gathers just those encoder rows
with indirect SWDGE DMAs (fp32->bf16 cast in flight, one 128-index call
per tile - the silicon-validated gather shape). Pad lanes are killed by
a host-built -1e10 compact-mask bias, so the math is exactly the
reference's masked softmax.

Per-core pipeline per batch row (9 compact 128-token tiles as chunks of
512/512/128):
  - encT built by PE transpose-mode matmuls (128x128 tiles) into PSUM
    (bf16), evacuated to SBUF with a fused bf16->fp8e4m3 cast on DVE
    (ScalarE fp8 casts measured noisier on silicon; GpSimd has no PSUM
    port).
  - e_proj transposed (d on partitions) with fp8 DoubleRow matmuls
    (K=256 per instruction): lhsT = host-prequantized w_enc * 64 fp8,
    rhs = encT fp8 pairs. The 1/64 rescale and (h_proj + a_b) ride the
    tanh activation's scale/bias; tanh runs on [128, 1024] tiles (chunk
    pairs) to halve the per-op ScalarE init cost.
  - scores = v . tanh as columns: N=1 matmuls, th 128x128 slices
    stationary, v column moving -> scoresT in a [128, 9] PSUM tile
    (accumulation groups strictly sequential per column - start=True
    clears has_written bank-wide).
  - softmax unnormalized: compact-mask bias added to scoresT PSUM, Exp
    on ScalarE with accum_out row-sums, denominator closed by one
    cross-partition N=1 matmul; the 1/sum rescale lands once on the
    final weighted sum.
  - weighted sum as N=1 matmuls: lhsT = natural-layout gathered rows
    (bf16, unquantized - fp8 enc here would put ~4% noise on the
    output), rhs = p column.
"""

import numpy as np
from contextlib import ExitStack

B, S, ENC, DEC = 32, 2048, 1024, 1024
N_CORES = 8
BC = B // N_CORES   # batch rows per core
W_SCALE = 64.0      # fp8 weight pre-scale (avoids e4m3 subnormal range)
# padded compact-token count: Binomial(2048, 0.5) is 1024 +- 22.6, so 1152
# is a +5.7 sigma bound (seed-0 data maxes at 1062)
P_PAD = 1152
CHUNKS = (512, 512, 128)   # compact tokens per chunk (= 9 tiles of 128)


def build_bass_kernel(bc=BC, s=S, e_dim=ENC, d_dim=DEC, debug=False):
    import concourse.bass as bass
    import concourse.tile as tile
    from concourse import bacc, mybir

    f32 = mybir.dt.float32
    bf16 = mybir.dt.bfloat16
    fp8 = mybir.dt.float8e4
    i32 = mybir.dt.int32
    u16 = mybir.dt.uint16
    Tanh = mybir.ActivationFunctionType.Tanh
    Exp = mybir.ActivationFunctionType.Exp
    DR = mybir.MatmulPerfMode.DoubleRow

    n_et = e_dim // 128            # e 128-tiles (contraction for e_proj)
    n_dt = d_dim // 128            # d 128-tiles (e_proj output tiles)
    n_gt = P_PAD // 128            # compact s-tiles per batch row (9)
    n_kk = n_et // 2               # DoubleRow K=256 steps
    # (chunk, tile-within-chunk) for each global compact tile
    tile_map = []
    for c, csz in enumerate(CHUNKS):
        for jj in range(csz // 128):
            tile_map.append((c, jj))

    nc = bacc.Bacc("TRN2", target_bir_lowering=False, debug=debug)

    hs_h = nc.dram_tensor("hidden_states", [bc, d_dim], f32, kind="ExternalInput")
    enc_h = nc.dram_tensor("encoder_outputs", [bc, s, e_dim], f32, kind="ExternalInput")
    gidx_h = nc.dram_tensor("gidx", [bc, 128, n_gt], i32, kind="ExternalInput")
    cbias_h = nc.dram_tensor("cbias", [bc, 128, n_gt], f32, kind="ExternalInput")
    ab_h = nc.dram_tensor("a_b", [d_dim], f32, kind="ExternalInput")
    vw_h = nc.dram_tensor("v_w", [d_dim], f32, kind="ExternalInput")
    wenc8_h = nc.dram_tensor("w_enc_fp8", [128, n_et, d_dim], fp8, kind="ExternalInput")
    wd8_h = nc.dram_tensor("w_dec_fp8", [128, n_dt, d_dim], fp8, kind="ExternalInput")
    id_h = nc.dram_tensor("ident", [128, 128], bf16, kind="ExternalInput")
    out_h = nc.dram_tensor("out", [bc, e_dim], f32, kind="ExternalOutput")

    enc_flat = enc_h[:, :, :].rearrange("b s e -> (b s) e")

    with tile.TileContext(nc) as tc, ExitStack() as ctx:
        consts = ctx.enter_context(tc.tile_pool(name="consts", bufs=1))
        enc_pool = ctx.enter_context(tc.tile_pool(name="enc", bufs=10))
        encT_pool = ctx.enter_context(tc.tile_pool(name="encT", bufs=4))
        th_pool = ctx.enter_context(tc.tile_pool(name="tanh", bufs=14))
        p_pool = ctx.enter_context(tc.tile_pool(name="p", bufs=2))
        small_pool = ctx.enter_context(tc.tile_pool(name="small", bufs=6))
        outsb_pool = ctx.enter_context(tc.tile_pool(name="outsb", bufs=2))
        pe_psum = ctx.enter_context(tc.tile_pool(name="pe_psum", bufs=2, space="PSUM"))
        tr_psum = ctx.enter_context(tc.tile_pool(name="tr_psum", bufs=2, space="PSUM"))
        sc_psum = ctx.enter_context(tc.tile_pool(name="sc_psum", bufs=1, space="PSUM"))
        w_psum = ctx.enter_context(tc.tile_pool(name="w_psum", bufs=1, space="PSUM"))

        # ---------------- prologue DMAs (transfers serialize; this order
        # is the pipeline-fill critical path) ----------------
        gidx_sb = consts.tile([128, bc, n_gt], i32)
        nc.sync.dma_start(out=gidx_sb, in_=gidx_h[:, :, :].rearrange("b p g -> p b g"))

        cbias_sb = consts.tile([128, bc, n_gt], f32)
        nc.sync.dma_start(
            out=cbias_sb, in_=cbias_h[:, :, :].rearrange("b p g -> p b g")
        )

        id_sb = consts.tile([128, 128], bf16)
        nc.sync.dma_start(out=id_sb, in_=id_h[:, :])

        hs_bf = consts.tile([bc, d_dim], bf16)
        nc.gpsimd.dma_start(out=hs_bf, in_=hs_h[:, :])  # cast f32->bf16

        enc_chunks = {}
        state = {}

        def emit_gather(b, c):
            """Gather unmasked encoder rows for chunk (b, c): one
            128-index SWDGE call per 128-token tile (the silicon-
            validated gather shape), f32->bf16 cast in the DMA."""
            enc_c = enc_pool.tile([128, 4, e_dim], bf16, tag="enc")
            g0 = sum(cs // 128 for cs in CHUNKS[:c])
            for jj in range(CHUNKS[c] // 128):
                nc.gpsimd.indirect_dma_start(
                    out=enc_c[:, jj, :],
                    out_offset=None,
                    in_=enc_flat,
                    in_offset=bass.IndirectOffsetOnAxis(
                        ap=gidx_sb[:, b, g0 + jj : g0 + jj + 1], axis=0
                    ),
                )
            enc_chunks[(b, c)] = enc_c

        emit_gather(0, 0)
        emit_gather(0, 1)
        emit_gather(0, 2)

        # weights in kk-pair slices: the DMA device serves transfers in
        # arrival order, and page-sized pieces interleave with the
        # batch-0 gather stream instead of blocking it for 6us
        wenc8_sb = consts.tile([128, n_et, d_dim], fp8)
        for kk in range(n_kk):
            nc.sync.dma_start(
                out=wenc8_sb[:, 2 * kk : 2 * kk + 2, :],
                in_=wenc8_h[:, 2 * kk : 2 * kk + 2, :],
            )

        wd8_sb = consts.tile([128, n_dt, d_dim], fp8)
        for kk in range(n_kk):
            nc.sync.dma_start(
                out=wd8_sb[:, 2 * kk : 2 * kk + 2, :],
                in_=wd8_h[:, 2 * kk : 2 * kk + 2, :],
            )

        emit_gather(1, 0)
        emit_gather(1, 1)
        emit_gather(1, 2)
        v_sb = consts.tile([128, n_dt], bf16)
        nc.gpsimd.dma_start(out=v_sb, in_=vw_h[:].rearrange("(i p) -> p i", p=128))

        ab_sb = consts.tile([128, n_dt], f32)
        nc.sync.dma_start(out=ab_sb, in_=ab_h[:].rearrange("(i p) -> p i", p=128))

        ones_col = consts.tile([128, 1], f32)
        nc.vector.memset(ones_col, 1.0)
        ones_row = consts.tile([1, 128], f32)
        nc.vector.memset(ones_row, 1.0)
        ones4 = consts.tile([128, bc], f32)
        nc.vector.memset(ones4, 1.0)
        # a_b broadcast to (d-tile, b) layout: ab_rep[p, i, :] = a_b[128i+p]
        ab_rep = consts.tile([128, n_dt, bc], f32)
        for i in range(n_dt):
            nc.vector.tensor_scalar_mul(ab_rep[:, i, :], ones4, ab_sb[:, i : i + 1])

        # ---------------- h_proj (tiny; emitted via mid-hook inside the
        # first e_proj so the in-order PE queue isn't head-blocked while
        # w_dec_fp8 is still in flight) ----------------
        hb_sb = consts.tile([128, n_dt, bc], f32)

        hproj_state = {}

        def emit_hproj_a():
            # hiddenT (d on partitions) via K=bc transpose-by-matmul,
            # emitted in the prologue: PE and DVE are otherwise idle
            # waiting for the first gathers, and this keeps the fp8 cast
            # ahead of the evacuation backlog in the in-order DVE queue.
            # PSUM comes from tr_psum: pe_psum buffers hold un-evacuated
            # e_proj output whose tanh waits on hb -> using them here
            # would deadlock the PE queue.
            psum_h = tr_psum.tile([128, n_dt * bc], f32, tag="tr")
            for k in range(n_dt):
                nc.tensor.matmul(
                    psum_h[:, bc * k : bc * (k + 1)],
                    lhsT=hs_bf[:, 128 * k : 128 * (k + 1)],
                    rhs=id_sb[0:bc, 0:bc],
                    start=True,
                    stop=True,
                )
            hT8 = consts.tile([128, n_dt, bc], fp8)
            nc.vector.tensor_copy(hT8, psum_h)
            hproj_state["hT8"] = hT8

        def emit_hproj():
            hT8 = hproj_state["hT8"]
            # single-PSUM accumulation: per-i-block groups run strictly
            # sequentially in one bank (start=True clears has_written
            # bank-wide but leaves data; closed blocks are never
            # re-accumulated)
            psum_hp = tr_psum.tile([128, n_dt * bc], f32, tag="tr")
            for i in range(n_dt):
                for k in range(n_dt):
                    nc.tensor.matmul(
                        psum_hp[:, bc * i : bc * (i + 1)],
                        lhsT=wd8_sb[:, k, 128 * i : 128 * (i + 1)],
                        rhs=hT8[:, k, :],
                        start=(k == 0),
                        stop=(k == n_dt - 1),
                    )
            # hb = psum / W_SCALE + a_b (weights were pre-scaled *64)
            nc.vector.scalar_tensor_tensor(
                hb_sb.rearrange("p a b -> p (a b)"),
                psum_hp,
                1.0 / W_SCALE,
                ab_rep.rearrange("p a b -> p (a b)"),
                op0=mybir.AluOpType.mult,
                op1=mybir.AluOpType.add,
            )

        # ---------------- per-chunk stages ----------------

        def emit_transpose_j(b, c, j):
            """One 128-token tile of encT for chunk (b, c): 8 PE
            transposes (all e-tiles of tile j) into a PSUM bank + one
            cast-evacuation (ScalarE where it would otherwise idle, DVE
            steady; GpSimd has no PSUM port). Per-tile units mean a unit
            only waits on its own gather op."""
            if (b, c) not in state:
                state[(b, c)] = encT_pool.tile(
                    [128, n_et, 512], fp8, tag="encT", name="encT8"
                )
            encT8 = state[(b, c)]
            chunk = enc_chunks[(b, c)]
            tp = tr_psum.tile([128, n_et, 128], bf16, tag="tr", name="tp")
            for et in range(n_et):
                nc.tensor.transpose(
                    tp[:, et, :],
                    chunk[:, j, 128 * et : 128 * (et + 1)],
                    id_sb,
                )
            dst = encT8[:, :, 128 * j : 128 * (j + 1)]
            nc.vector.tensor_copy(dst, tp)

        def emit_transposes(b, c):
            for j in range(CHUNKS[c] // 128):
                emit_transpose_j(b, c, j)

        def emit_eproj_pair(b, mid_hook=None):
            # chunks 0+1 together: tanh runs on [128, 1024] tiles (one
            # ScalarE init per two chunks); the two 512-wide matmul
            # groups land in the two banks of a 2-bank PSUM tile.
            eTa = state.pop((b, 0))
            eTb = state.pop((b, 1))
            state[("sc", b)] = sc_psum.tile([128, n_gt], f32, tag="sc", name="psc")
            if mid_hook is not None:
                mid_hook()
                mid_hook = None
            ths = []
            for i in range(n_dt):
                pe = pe_psum.tile([128, 2, 512], f32, tag="pe")
                for half, eT in ((0, eTa), (1, eTb)):
                    for kk in range(n_kk):
                        nc.tensor.matmul(
                            pe[:, half, :],
                            lhsT=wenc8_sb[
                                :, 2 * kk : 2 * kk + 2, 128 * i : 128 * (i + 1)
                            ],
                            rhs=eT[:, 2 * kk : 2 * kk + 2, :],
                            start=(kk == 0),
                            stop=(kk == n_kk - 1),
                            perf_mode=DR,
                        )
                th = th_pool.tile([128, 2, 512], bf16, tag="tanh")
                nc.scalar.activation(
                    th.rearrange("p a b -> p (a b)"),
                    pe.rearrange("p a b -> p (a b)"),
                    Tanh,
                    bias=hb_sb[:, i, b : b + 1],
                    scale=1.0 / W_SCALE,
                )
                ths.append(th)
            state[("th", b)] = ths

        def emit_eproj_tail(b):
            # chunk 2: single 128-token tile
            eT = state.pop((b, 2))
            ths = []
            for i in range(n_dt):
                pe = pe_psum.tile([128, 2, 512], f32, tag="pe")
                for kk in range(n_kk):
                    nc.tensor.matmul(
                        pe[:, 0, 0:128],
                        lhsT=wenc8_sb[:, 2 * kk : 2 * kk + 2, 128 * i : 128 * (i + 1)],
                        rhs=eT[:, 2 * kk : 2 * kk + 2, 0:128],
                        start=(kk == 0),
                        stop=(kk == n_kk - 1),
                        perf_mode=DR,
                    )
                th = th_pool.tile([128, 128], bf16, tag="ttail")
                nc.scalar.activation(
                    th, pe[:, 0, 0:128], Tanh, bias=hb_sb[:, i, b : b + 1],
                    scale=1.0 / W_SCALE,
                )
                ths.append(th)
            state[("tht", b)] = ths

        scores_done = {}

        def emit_eproj_single(b, c):
            # 512-wide e_proj for one chunk: batch 0 only, so the first
            # e_proj/tanh start as soon as ONE chunk is evacuated instead
            # of two (the pipeline-fill critical path).
            eT = state.pop((b, c))
            if c == 0:
                state[("sc", b)] = sc_psum.tile(
                    [128, n_gt], f32, tag="sc", name="psc"
                )
                state[("th", b)] = ("split", [[], []])
            ths_c = state[("th", b)][1][c]
            for i in range(n_dt):
                pe = pe_psum.tile([128, 2, 512], f32, tag="pe")
                for kk in range(n_kk):
                    nc.tensor.matmul(
                        pe[:, 0, :],
                        lhsT=wenc8_sb[:, 2 * kk : 2 * kk + 2, 128 * i : 128 * (i + 1)],
                        rhs=eT[:, 2 * kk : 2 * kk + 2, :],
                        start=(kk == 0),
                        stop=(kk == n_kk - 1),
                        perf_mode=DR,
                    )
                th = th_pool.tile([128, 512], bf16, tag="tanh_s")
                nc.scalar.activation(
                    th, pe[:, 0, :], Tanh, bias=hb_sb[:, i, b : b + 1],
                    scale=1.0 / W_SCALE,
                )
                ths_c.append(th)

        def emit_scores_col(b, col):
            # Column-outer, i-inner: accumulation groups in the scoresT
            # bank must be strictly sequential (start=True clears
            # has_written for the WHOLE bank).
            ths = state[("th", b)]
            psum_sc = state[("sc", b)]
            half, jj = divmod(col, 4)
            for i in range(n_dt):
                if isinstance(ths, tuple):
                    lhsT = ths[1][half][i][:, 128 * jj : 128 * (jj + 1)]
                else:
                    lhsT = ths[i][:, half, 128 * jj : 128 * (jj + 1)]
                nc.tensor.matmul(
                    psum_sc[:, col : col + 1],
                    lhsT=lhsT,
                    rhs=v_sb[:, i : i + 1],
                    start=(i == 0),
                    stop=(i == n_dt - 1),
                )
            scores_done[b] = scores_done.get(b, 0) + 1

        def emit_scores_pair(b):
            for col in range(scores_done.get(b, 0), 8):
                emit_scores_col(b, col)
            state.pop(("th", b))

        def emit_scores_tail(b):
            ths = state.pop(("tht", b))
            psum_sc = state[("sc", b)]
            for i in range(n_dt):
                nc.tensor.matmul(
                    psum_sc[:, 8:9],
                    lhsT=ths[i],
                    rhs=v_sb[:, i : i + 1],
                    start=(i == 0),
                    stop=(i == n_dt - 1),
                )

        def emit_softmax_a(b):
            """Compact-mask bias + exp with fused row-sums (DVE+ScalarE)."""
            psum_sc = state.pop(("sc", b))
            nc.vector.tensor_add(psum_sc, psum_sc, cbias_sb[:, b, :])
            p_bf = p_pool.tile([128, n_gt], bf16, tag="p")
            rowsum = small_pool.tile([128, 1], f32, tag="rowsum")
            nc.scalar.activation(
                p_bf, psum_sc, Exp, bias=0.0, scale=1.0, accum_out=rowsum
            )
            state[("p", b)] = p_bf
            state[("rowsum", b)] = rowsum

        def emit_ssum_recip(b):
            rowsum = state.pop(("rowsum", b))
            ssum = w_psum.tile([1, 1], f32, tag="w")
            nc.tensor.matmul(ssum, lhsT=rowsum, rhs=ones_col, start=True, stop=True)
            rsum = small_pool.tile([1, 1], f32, tag="rsum")
            nc.vector.reciprocal(rsum, ssum)
            state[("rsum", b)] = rsum

        def emit_weighted(b):
            p_bf = state.pop(("p", b))
            rsum = state.pop(("rsum", b))
            rbc_ps = w_psum.tile([128, 1], f32, tag="w")
            nc.tensor.matmul(rbc_ps, lhsT=ones_row, rhs=rsum, start=True, stop=True)
            rbc = small_pool.tile([128, 1], f32, tag="rbc")
            nc.vector.tensor_copy(rbc, rbc_ps)
            w_ps = w_psum.tile([128, n_dt], f32, tag="w")
            for i in range(n_et):
                for g, (c, jj) in enumerate(tile_map):
                    nc.tensor.matmul(
                        w_ps[:, i : i + 1],
                        lhsT=enc_chunks[(b, c)][:, jj, 128 * i : 128 * (i + 1)],
                        rhs=p_bf[:, g : g + 1],
                        start=(g == 0),
                        stop=(g == n_gt - 1),
                    )
            for c in range(len(CHUNKS)):
                del enc_chunks[(b, c)]
            out_sb = outsb_pool.tile([128, n_et], f32, tag="outsb")
            nc.vector.tensor_scalar_mul(out_sb, w_ps, rbc[:, 0:1])
            nc.sync.dma_start(
                out=out_h[b, :].rearrange("(i p) -> p i", p=128), in_=out_sb
            )

        # ---------------- schedule ----------------
        # Two sub-stages per batch row: A(b) = chunks 0+1 e_proj, B(b) =
        # tail e_proj. Transposes run one sub-stage ahead of their
        # e_proj, scores one sub-stage behind, so the in-order PE queue
        # never blocks on ScalarE/DVE results.
        emit_hproj_a()
        emit_transposes(0, 0)
        emit_transposes(0, 1)
        for b in range(bc):
            # --- sub-stage A(b) ---
            if b + 2 < bc:
                emit_gather(b + 2, 0)
                emit_gather(b + 2, 1)
            if b > 0:
                emit_scores_tail(b - 1)
                emit_softmax_a(b - 1)
            if b == 0:
                emit_hproj()
                emit_eproj_single(0, 0)
                emit_eproj_single(0, 1)
            else:
                emit_eproj_pair(b)
            emit_transposes(b, 2)
            # --- sub-stage B(b) ---
            if b + 2 < bc:
                emit_gather(b + 2, 2)
            emit_eproj_tail(b)
            if b + 1 < bc:
                emit_transposes(b + 1, 0)
                emit_transposes(b + 1, 1)
            emit_scores_pair(b)
            if b > 0:
                emit_ssum_recip(b - 1)
                emit_weighted(b - 1)
        emit_scores_tail(bc - 1)
        emit_softmax_a(bc - 1)
        emit_ssum_recip(bc - 1)
        emit_weighted(bc - 1)

    nc.compile()
    return nc


_CACHE = {}


def _prep_weights(a_w):
    """Host-side weight repack: w_enc and w_dec scaled by 64 and
    quantized to fp8e4m3 in (p, k, d) layout matching the stationary-
    operand slices (DoubleRow pairs for w_enc)."""
    import ml_dtypes

    def pack(w):
        w = (np.asarray(w, dtype=np.float32) * W_SCALE).reshape(-1, 128, DEC)
        return np.ascontiguousarray(w.transpose(1, 0, 2)).astype(
            ml_dtypes.float8_e4m3
        )

    return pack(a_w[DEC:]), pack(a_w[:DEC])


def _prep_indices(masks):
    """Per-row unmasked token indices (padded to P_PAD with row 0 of the
    same batch row - its lanes are killed by cbias) and the compact-mask
    bias, both in column-major (p, g) tile layout."""
    bc = masks.shape[0]
    gidx = np.zeros((bc, P_PAD), dtype=np.int32)
    cbias = np.full((bc, P_PAD), -1e10, dtype=np.float32)
    for b in range(bc):
        idx = np.nonzero(masks[b])[0].astype(np.int32)
        cnt = len(idx)
        assert cnt <= P_PAD, f"unmasked count {cnt} exceeds P_PAD={P_PAD}"
        gidx[b, :cnt] = b * S + idx
        gidx[b, cnt:] = b * S
        cbias[b, :cnt] = 0.0
    # (b, tile*128 + p) -> (b, p, tile)
    gidx = np.ascontiguousarray(gidx.reshape(bc, P_PAD // 128, 128).transpose(0, 2, 1))
    cbias = np.ascontiguousarray(
        cbias.reshape(bc, P_PAD // 128, 128).transpose(0, 2, 1)
    )
    return gidx, cbias


def kernel(hidden_states, encoder_outputs, encoder_masks, a_w, a_b, v_w):
    import ml_dtypes
    from concourse.bass_utils import run_bass_kernel_spmd

    if "nc" not in _CACHE:
        _CACHE["nc"] = build_bass_kernel()
    nc = _CACHE["nc"]

    hidden_states = np.asarray(hidden_states, dtype=np.float32)
    encoder_outputs = np.asarray(encoder_outputs, dtype=np.float32)
    encoder_masks = np.asarray(encoder_masks, dtype=np.int32)
    a_w = np.ascontiguousarray(np.asarray(a_w, dtype=np.float32))
    a_b = np.ascontiguousarray(np.asarray(a_b, dtype=np.float32))
    v_w = np.ascontiguousarray(np.asarray(v_w, dtype=np.float32))
    ident = np.eye(128, dtype=ml_dtypes.bfloat16)
    wenc8, wd8 = _prep_weights(a_w)

    in_maps = []
    for c in range(N_CORES):
        sl = slice(c * BC, (c + 1) * BC)
        gidx, cbias = _prep_indices(encoder_masks[sl])
        in_maps.append(
            {
                "hidden_states": np.ascontiguousarray(hidden_states[sl]),
                "encoder_outputs": np.ascontiguousarray(encoder_outputs[sl]),
                "gidx": gidx,
                "cbias": cbias,
                "a_b": a_b,
                "v_w": v_w,
                "w_enc_fp8": wenc8,
                "w_dec_fp8": wd8,
                "ident": ident,
            }
        )

    global _LAST_IN_MAPS
    _LAST_IN_MAPS = in_maps
    res = run_bass_kernel_spmd(nc, in_maps, core_ids=list(range(N_CORES)))
    out = np.concatenate([r["out"] for r in res.results], axis=0)
    return out.astype(np.float32)


_LAST_IN_MAPS = None
